# revision 36
# baseline (speedup 1.0000x reference)
"""DelayLMLIFLayer Trainium2 kernel.

Pipeline per core (8 cores, 4-way I-shard x 2-way B-shard):
  1. Pass 1: DCLS delayed conv main term as 16 time-shifted PSUM-accumulated
     f32r matmuls per chunk; ACT drains PSUM into At (y_main) while
     accumulating BN sum/sum-of-squares per chunk. Startup DMAs are split
     across the SP and ACT queues so the first matmul is gated by
     max(weights, x) rather than their sum.
  2. BN stats: pairwise AllGather (b-half pairs) + local add; fold BN affine,
     (1-beta) input scale, and the scan's -beta constant into per-channel
     a, b'. The post-collective DMA rides the ACT queue (the SP queue would
     stall pass-2 x DMAs behind the collective wait), and the fold is emitted
     mid-pass-2 so its ACT sqrt doesn't re-serialize the PSUM copies.
  3. Pass 2 (races the scan): bf16 cross terms (xh@wl + xl@wh) -> ACT copy,
     Pool add into At, ACT affine At = a*At + b' (per-partition scale/bias;
     chunk 0 in halves so the scan starts sooner).
  4. LIF scan on DVE, 2 ops/step/chain (2 chains of 8 batch): with
     W' := U - S + 1 (host seeds W'_0 = U0 + 1),
       U_t  = beta*W'_{t-1} + A'_t         (A' = a*y + b', b' folds -beta)
       W'_t = (U_t < 1) + U_t
     Spikes leave the critical chain entirely: S = (U >= 1) is computed
     chunk-wise in bulk on Pool from the stored U history, written over At,
     then DMA'd out (last chunk in quarters to shorten the tail).
Host does layout transposes and the fp32r/bf16 splits; device time is what
counts. 490us predicted vs the 663us 3-op-scan baseline.
"""
import sys
sys.path.insert(0, '/opt/trn_rl_repo')

import numpy as np

T, B, J, I, KD = 1024, 32, 128, 512, 16
SIG = 0.5
EPS = 1e-5
N_CORES = 8
BH = B // 2          # batch elems per core (b-half)
IC = 128             # channels per core (I-chunk)
ROWS = T * BH        # free-dim rows per core
PAD = (KD - 1) * BH  # left zero pad columns (240)
CHUNK = 512          # psum tile free size
NCH = ROWS // CHUNK  # 32 row chunks
TPC = CHUNK // BH    # 32 timesteps per chunk

_CACHE = {}


def _to_fp32r(x):
    u = np.ascontiguousarray(x, np.float32).view(np.uint32).astype(np.uint64)
    rnd = ((u >> 12) & 1) + 0x7FF
    u = ((u + rnd) >> 12) << 12
    return (u & 0xFFFFFFFF).astype(np.uint32).view(np.float32)


def _build_nc():
    import concourse.bacc as bacc
    import concourse.mybir as mybir
    import concourse.tile as tile

    F32 = mybir.dt.float32
    F32R = mybir.dt.float32r
    BF16 = mybir.dt.bfloat16
    OP = mybir.AluOpType
    AF = mybir.ActivationFunctionType

    nc = bacc.Bacc("TRN2", target_bir_lowering=False, debug=False,
                   num_devices=N_CORES)

    xh_d = nc.dram_tensor("xh", [J, ROWS], F32, kind="ExternalInput")
    xhb_d = nc.dram_tensor("xhb", [J, ROWS], BF16, kind="ExternalInput")
    xlb_d = nc.dram_tensor("xlb", [J, ROWS], BF16, kind="ExternalInput")
    wh_d = nc.dram_tensor("wh", [J, KD, IC], F32, kind="ExternalInput")
    whb_d = nc.dram_tensor("whb", [KD, J, IC], BF16, kind="ExternalInput")
    wlb_d = nc.dram_tensor("wlb", [KD, J, IC], BF16, kind="ExternalInput")
    u0_d = nc.dram_tensor("u0", [IC, BH], F32, kind="ExternalInput")
    pch_d = nc.dram_tensor("pch", [IC, 3], F32, kind="ExternalInput")
    sout_d = nc.dram_tensor("sout", [IC, ROWS], F32, kind="ExternalOutput")

    with tile.TileContext(nc) as tc:
        with (
            tc.tile_pool(name="big", bufs=1) as big,
            tc.tile_pool(name="xs", bufs=3) as xs,
            tc.tile_pool(name="small", bufs=1) as small,
            tc.tile_pool(name="ps", bufs=4, space="PSUM") as ps,
            tc.tile_pool(name="dram", bufs=1, space="DRAM") as dram,
        ):
            At = [big.tile([IC, CHUNK], F32, tag=f"A{r}", name=f"A{r}")
                  for r in range(NCH)]
            Ut = [big.tile([IC, CHUNK], F32, tag=f"U{r}", name=f"U{r}")
                  for r in range(NCH)]
            scr = big.tile([IC, CHUNK], F32, tag="scr")
            whg = [small.tile([J, 4, IC], F32R, tag=f"whg{g}", name=f"whg{g}")
                   for g in range(4)]
            whb = small.tile([J, KD, IC], BF16, tag="whb")
            wlb = small.tile([J, KD, IC], BF16, tag="wlb")
            pch = small.tile([IC, 3], F32, tag="pch")
            Wc = small.tile([IC, BH], F32, tag="Wc")
            ssum = small.tile([IC, NCH], F32, tag="ssum")
            ssq = small.tile([IC, NCH], F32, tag="ssq")
            st2 = small.tile([IC, 2], F32, tag="st2")
            gs = small.tile([IC, 2], F32, tag="gs")
            prm = small.tile([IC, 8], F32, tag="prm")

            cc_in = dram.tile([IC, 2], F32)
            cc_out = dram.tile([2, IC, 2], F32)

            # startup: weights grouped 4 taps per DMA, split across the SP
            # and ACT queues so they land just ahead of tap consumption; the
            # first x slice rides SP first (tap 0 only needs the pad memset
            # + 272-col slice).
            xh_c0 = xs.tile([J, PAD + CHUNK], F32R, tag="xh_c")
            nc.vector.memset(xh_c0[:, :PAD].bitcast(F32), 0.0)
            nc.sync.dma_start(xh_c0[:, PAD:PAD + 272], xh_d[:, 0:272].bitcast(F32R))
            nc.sync.dma_start(whg[0][:], wh_d[:, 0:4, :].bitcast(F32R))
            nc.sync.dma_start(xh_c0[:, PAD + 272:], xh_d[:, 272:CHUNK].bitcast(F32R))
            nc.sync.dma_start(whg[2][:], wh_d[:, 8:12, :].bitcast(F32R))
            nc.scalar.dma_start(whg[1][:], wh_d[:, 4:8, :].bitcast(F32R))
            nc.scalar.dma_start(whg[3][:], wh_d[:, 12:16, :].bitcast(F32R))
            nc.scalar.dma_start(Wc[:], u0_d[:])     # host sends W'_0 = U0 + 1
            nc.scalar.dma_start(pch[:], pch_d[:])
            # Dummy sqrt so the act-table pass picks sqrt_and_friends (the
            # one set holding Copy/Square/Sqrt/Identity) at t=0 instead of
            # reloading tables right before the first affine.
            nc.vector.memset(prm[:], 0.0)
            nc.scalar.sqrt(prm[:, 7:8], prm[:, 6:7])
            # onemb = 1 - beta depends only on pch: compute off the
            # post-collective critical path.
            nc.vector.tensor_scalar(prm[:, 4:5], pch[:, 0:1], -1.0, 1.0,
                                    OP.mult, OP.add)
            # PE p-state warmup: dummy matmuls on the zeroed pad region while
            # the first weights are still in flight, so the real pass-1
            # matmuls start at full clock (model needs ~3us of PE busy).
            ptd = ps.tile([IC, PAD], F32, tag="pt")
            for _ in range(6):
                nc.tensor.matmul(ptd[:], xh_c0[:, 0:IC], xh_c0[:, 0:PAD],
                                 start=True, stop=True)
            beta = pch[:, 0:1]
            gamma = pch[:, 1:2]
            bnbeta = pch[:, 2:3]

            # ---- conv pass 1: main fp32r term; doubles as the BN stats source ----
            for r in range(NCH):
                c0 = r * CHUNK - PAD
                if r == 0:
                    xh_c = xh_c0
                else:
                    xh_c = xs.tile([J, PAD + CHUNK], F32R, tag="xh_c")
                    nc.sync.dma_start(xh_c[:], xh_d[:, c0:c0 + PAD + CHUNK].bitcast(F32R))

                pt = ps.tile([IC, CHUNK], F32, tag="pt")
                for k in range(KD):
                    nc.tensor.matmul(pt[:], whg[k // 4][:, k % 4, :],
                                     xh_c[:, k * BH:k * BH + CHUNK],
                                     start=(k == 0), stop=(k == KD - 1))

                if r < NCH - 1:
                    nc.scalar.activation(At[r][:], pt[:], AF.Copy,
                                         accum_out=ssum[:, r:r + 1])
                else:
                    # last chunk: skip the Copy's accumulator drain (it
                    # serializes the Square behind a 187ns readback); DVE
                    # reduces the row sum from SBUF instead.
                    nc.scalar.activation(At[r][:], pt[:], AF.Copy)
                    nc.vector.tensor_reduce(ssum[:, r:r + 1], At[r][:],
                                            mybir.AxisListType.X, OP.add)
                nc.scalar.activation(scr[:], pt[:], AF.Square,
                                     accum_out=ssq[:, r:r + 1])

            # ---- BN stats allreduce over the b-half pair ----
            nc.vector.tensor_reduce(st2[:, 0:1], ssum[:], mybir.AxisListType.X, OP.add)
            nc.vector.tensor_reduce(st2[:, 1:2], ssq[:], mybir.AxisListType.X, OP.add)
            nc.sync.dma_start(cc_in[:], st2[:])
            # AllGather + local add: same result as AllReduce (order-proof
            # since add is commutative) at roughly half the fixed latency.
            nc.gpsimd.collective_compute(
                "AllGather", OP.bypass,
                replica_groups=[[0, 1], [2, 3], [4, 5], [6, 7]],
                ins=[cc_in.opt()], outs=[cc_out.opt()],
            )
            # On the ACT queue: a sync-queue DMA here would wait on the
            # collective semaphore and stall every pass-2 x DMA behind it.
            # ACT's own downstream (the affine) waits on the fold anyway.
            gs4 = small.tile([IC, 4], F32, tag="gs4")
            nc.scalar.dma_start(gs4[:].rearrange("p (g s) -> p g s", g=2),
                                cc_out[:, :, :].transpose([1, 0, 2]))

            inv_n = 1.0 / (T * B)
            mean = prm[:, 0:1]; ey2 = prm[:, 1:2]; var = prm[:, 2:3]
            inv = prm[:, 3:4]; onemb = prm[:, 4:5]; av = prm[:, 5:6]
            bv = prm[:, 6:7]; tmp = prm[:, 7:8]

            def fold_block():
                # fold BN + (1-beta) + scan's -beta into per-channel a, b'.
                # Emitted after a few pass-2 copies so the ACT sqrt doesn't
                # re-serialize them behind the collective. onemb was computed
                # at startup.
                nc.vector.tensor_tensor(gs[:], gs4[:, 0:2], gs4[:, 2:4], OP.add)
                nc.vector.tensor_scalar(mean, gs[:, 0:1], inv_n, None, OP.mult)
                nc.vector.tensor_scalar(ey2, gs[:, 1:2], inv_n, EPS,
                                        OP.mult, OP.add)    # E[y^2] + eps
                # var_neg = mean^2 - (E[y^2]+eps); sqrt applies scale=-1
                nc.vector.scalar_tensor_tensor(var, mean, mean, ey2,
                                               OP.mult, OP.subtract)
                nc.scalar.activation(tmp, var, AF.Sqrt, scale=-1.0)
                nc.vector.reciprocal(inv, tmp)
                nc.vector.tensor_tensor(inv, gamma, inv, OP.mult)   # gamma*rsqrt
                nc.vector.tensor_tensor(av, onemb, inv, OP.mult)    # a = (1-b)*g*rsqrt
                nc.vector.tensor_tensor(tmp, inv, mean, OP.mult)
                nc.vector.tensor_tensor(tmp, bnbeta, tmp, OP.subtract)
                nc.vector.tensor_tensor(bv, onemb, tmp, OP.mult)    # (1-b)*(bn_b - g*rsqrt*mean)
                nc.vector.tensor_tensor(bv, bv, beta, OP.subtract)  # b' = b - beta

            # ---- conv pass 2: bf16 cross terms + affine, racing the scan ----
            for k in range(KD):
                nc.sync.dma_start(whb[:, k, :], whb_d[k, :, :])
                nc.sync.dma_start(wlb[:, k, :], wlb_d[k, :, :])
            FOLD_AT = 3
            for r in range(NCH):
                c0 = r * CHUNK - PAD
                xhb_c = xs.tile([J, PAD + CHUNK], BF16, tag="xhb_c")
                xlb_c = xs.tile([J, PAD + CHUNK], BF16, tag="xlb_c")
                if r == 0:
                    nc.vector.memset(xhb_c[:, :PAD], 0.0)
                    nc.vector.memset(xlb_c[:, :PAD], 0.0)
                    nc.sync.dma_start(xhb_c[:, PAD:], xhb_d[:, 0:CHUNK])
                    nc.sync.dma_start(xlb_c[:, PAD:], xlb_d[:, 0:CHUNK])
                else:
                    nc.sync.dma_start(xhb_c[:], xhb_d[:, c0:c0 + PAD + CHUNK])
                    nc.sync.dma_start(xlb_c[:], xlb_d[:, c0:c0 + PAD + CHUNK])

                pt2 = ps.tile([IC, CHUNK], F32, tag="pt2")
                for k in range(KD):
                    nc.tensor.matmul(pt2[:], wlb[:, k, :], xhb_c[:, k * BH:k * BH + CHUNK],
                                     start=(k == 0), stop=False)
                for k in range(KD):
                    nc.tensor.matmul(pt2[:], whb[:, k, :], xlb_c[:, k * BH:k * BH + CHUNK],
                                     start=False, stop=(k == KD - 1))

                s2 = xs.tile([IC, CHUNK], F32, tag="s2")
                sl = At[r][:]
                nc.scalar.activation(s2[:], pt2[:], AF.Copy)
                nc.gpsimd.tensor_tensor(sl, sl, s2[:], OP.add)
                # A' = a*y + b' in one ACT op (per-partition scale/bias)
                if r == FOLD_AT:
                    fold_block()
                    # chunk 0's affine in halves so the scan can start on the
                    # first 16 steps while the rest is still being scaled
                    HC = CHUNK // 2
                    nc.scalar.activation(At[0][:, :HC], At[0][:, :HC],
                                         AF.Identity, bias=bv, scale=av)
                    nc.scalar.activation(At[0][:, HC:], At[0][:, HC:],
                                         AF.Identity, bias=bv, scale=av)
                    for rr in range(1, FOLD_AT + 1):
                        nc.scalar.activation(At[rr][:], At[rr][:], AF.Identity,
                                             bias=bv, scale=av)
                elif r > FOLD_AT:
                    nc.scalar.activation(sl, sl, AF.Identity, bias=bv, scale=av)

            # ---- LIF scan: 2 DVE ops per step per chain, spikes off-chain ----
            # W' = U - S + 1 lets the reset fold into one scalar_tensor_tensor:
            #   U  = beta*W' + A'     (b' above already absorbed the -beta)
            #   W' = (U < 1) + U
            HB = BH // 2
            for t in range(T):
                rt, lt = t // TPC, (t % TPC) * BH
                a0 = At[rt][:, lt:lt + HB]
                a1 = At[rt][:, lt + HB:lt + BH]
                u0_ = Ut[rt][:, lt:lt + HB]
                u1_ = Ut[rt][:, lt + HB:lt + BH]
                w0_, w1_ = Wc[:, :HB], Wc[:, HB:]
                nc.vector.scalar_tensor_tensor(u0_, w0_, beta, a0, OP.mult, OP.add)
                nc.vector.scalar_tensor_tensor(u1_, w1_, beta, a1, OP.mult, OP.add)
                nc.vector.scalar_tensor_tensor(w0_, u0_, 1.0, u0_, OP.is_lt, OP.add)
                nc.vector.scalar_tensor_tensor(w1_, u1_, 1.0, u1_, OP.is_lt, OP.add)

            # ---- bulk spike extraction on Pool + DMA out ----
            for r in range(NCH):
                if r < NCH - 1:
                    nc.gpsimd.tensor_scalar(At[r][:], Ut[r][:], 1.0, None, OP.is_ge)
                    nc.sync.dma_start(sout_d[:, r * CHUNK:(r + 1) * CHUNK], At[r][:])
                else:
                    # last chunk in shrinking slices so the final DMA after
                    # step 1023 is minimal
                    bnds = [0, 128, 256, 384, 448, 480, 496, 512]
                    for q in range(len(bnds) - 1):
                        a, b = r * CHUNK + bnds[q], r * CHUNK + bnds[q + 1]
                        la, lb = bnds[q], bnds[q + 1]
                        nc.gpsimd.tensor_scalar(At[r][:, la:lb],
                                                Ut[r][:, la:lb],
                                                1.0, None, OP.is_ge)
                        nc.sync.dma_start(sout_d[:, a:b], At[r][:, la:lb])

    nc.finalize()
    return nc


def _prep_inputs(x, delay_w, delay_P, beta, bn_gamma, bn_beta, U0):
    import ml_dtypes
    c = (delay_P.astype(np.float32) + KD // 2)
    k = np.arange(KD, dtype=np.float32)
    g = np.exp(-0.5 * ((k[None, None, :] - c[:, :, None]) / SIG) ** 2).astype(np.float32)
    g = g / (g.sum(-1, keepdims=True) + np.float32(1e-7))
    kern = (delay_w.astype(np.float32)[:, :, None] * g).astype(np.float32)  # (I,J,KD)

    kh = _to_fp32r(kern)
    kl = (kern - kh).astype(np.float32)
    xh = _to_fp32r(x)
    xl = (x - xh).astype(np.float32)

    wt_h = np.ascontiguousarray(kh.transpose(2, 1, 0))                     # (KD,J,I) f32
    wt_hb = wt_h.astype(ml_dtypes.bfloat16)
    wt_lb = np.ascontiguousarray(kl.transpose(2, 1, 0)).astype(ml_dtypes.bfloat16)
    wt_hj = np.ascontiguousarray(kh.transpose(1, 2, 0))                    # (J,KD,I) f32

    xt_h = np.ascontiguousarray(xh.transpose(2, 0, 1))                     # (J,T,B) f32
    xt_hb = xt_h.astype(ml_dtypes.bfloat16)
    xt_lb = np.ascontiguousarray(xl.transpose(2, 0, 1)).astype(ml_dtypes.bfloat16)

    in_maps = []
    for core in range(N_CORES):
        gi, hi = core // 2, core % 2
        isl = slice(gi * IC, (gi + 1) * IC)
        bsl = slice(hi * BH, (hi + 1) * BH)
        pch = np.stack([beta[isl], bn_gamma[isl], bn_beta[isl]], axis=1)
        in_maps.append({
            "xh": np.ascontiguousarray(xt_h[:, :, bsl]).reshape(J, ROWS),
            "xhb": np.ascontiguousarray(xt_hb[:, :, bsl]).reshape(J, ROWS),
            "xlb": np.ascontiguousarray(xt_lb[:, :, bsl]).reshape(J, ROWS),
            "wh": np.ascontiguousarray(wt_hj[:, :, isl]),
            "whb": np.ascontiguousarray(wt_hb[:, :, isl]),
            "wlb": np.ascontiguousarray(wt_lb[:, :, isl]),
            "u0": np.ascontiguousarray(U0[bsl, isl].T) + np.float32(1.0),
            "pch": np.ascontiguousarray(pch.astype(np.float32)),
        })
    return in_maps


def run_spmd(in_maps, **kwargs):
    from concourse.bass_utils import run_bass_kernel_spmd
    if "nc" not in _CACHE:
        _CACHE["nc"] = _build_nc()
    return run_bass_kernel_spmd(_CACHE["nc"], in_maps,
                                core_ids=list(range(N_CORES)), **kwargs)


def kernel(x, delay_w, delay_P, beta, bn_gamma, bn_beta, U0):
    in_maps = _prep_inputs(np.asarray(x, np.float32), np.asarray(delay_w, np.float32),
                           np.asarray(delay_P, np.float32), np.asarray(beta, np.float32),
                           np.asarray(bn_gamma, np.float32), np.asarray(bn_beta, np.float32),
                           np.asarray(U0, np.float32))
    res = run_spmd(in_maps)
    out = np.empty((T, B, I), np.float32)
    for core in range(N_CORES):
        gi, hi = core // 2, core % 2
        s = res.results[core]["sout"].reshape(IC, T, BH)
        out[:, hi * BH:(hi + 1) * BH, gi * IC:(gi + 1) * IC] = s.transpose(1, 2, 0)
    return out


# revision 37
# speedup vs baseline: 1.0025x; 1.0025x over previous
"""DelayLMLIFLayer Trainium2 kernel.

Pipeline per core (8 cores, 4-way I-shard x 2-way B-shard):
  1. Pass 1: DCLS delayed conv main term as 16 time-shifted PSUM-accumulated
     f32r matmuls per chunk; ACT drains PSUM into At (y_main) while
     accumulating BN sum/sum-of-squares per chunk. Startup DMAs are split
     across the SP and ACT queues so the first matmul is gated by
     max(weights, x) rather than their sum.
  2. BN stats: pairwise AllGather (b-half pairs) + local add; fold BN affine,
     (1-beta) input scale, and the scan's -beta constant into per-channel
     a, b'. The post-collective DMA rides the ACT queue (the SP queue would
     stall pass-2 x DMAs behind the collective wait), and the fold is emitted
     mid-pass-2 so its ACT sqrt doesn't re-serialize the PSUM copies.
  3. Pass 2 (races the scan): bf16 cross terms (xh@wl + xl@wh) -> ACT copy,
     Pool add into At, ACT affine At = a*At + b' (per-partition scale/bias;
     chunk 0 in halves so the scan starts sooner).
  4. LIF scan on DVE, 2 ops/step/chain (2 chains of 8 batch): with
     W' := U - S + 1 (host seeds W'_0 = U0 + 1),
       U_t  = beta*W'_{t-1} + A'_t         (A' = a*y + b', b' folds -beta)
       W'_t = (U_t < 1) + U_t
     Spikes leave the critical chain entirely: S = (U >= 1) is computed
     chunk-wise in bulk on Pool from the stored U history, written over At,
     then DMA'd out (last chunk in quarters to shorten the tail).
Host does layout transposes and the fp32r/bf16 splits; device time is what
counts. 490us predicted vs the 663us 3-op-scan baseline.
"""
import sys
sys.path.insert(0, '/opt/trn_rl_repo')

import numpy as np

T, B, J, I, KD = 1024, 32, 128, 512, 16
SIG = 0.5
EPS = 1e-5
N_CORES = 8
BH = B // 2          # batch elems per core (b-half)
IC = 128             # channels per core (I-chunk)
ROWS = T * BH        # free-dim rows per core
PAD = (KD - 1) * BH  # left zero pad columns (240)
CHUNK = 512          # psum tile free size
NCH = ROWS // CHUNK  # 32 row chunks
TPC = CHUNK // BH    # 32 timesteps per chunk

_CACHE = {}


def _to_fp32r(x):
    u = np.ascontiguousarray(x, np.float32).view(np.uint32).astype(np.uint64)
    rnd = ((u >> 12) & 1) + 0x7FF
    u = ((u + rnd) >> 12) << 12
    return (u & 0xFFFFFFFF).astype(np.uint32).view(np.float32)


def _build_nc():
    import concourse.bacc as bacc
    import concourse.mybir as mybir
    import concourse.tile as tile

    F32 = mybir.dt.float32
    F32R = mybir.dt.float32r
    BF16 = mybir.dt.bfloat16
    OP = mybir.AluOpType
    AF = mybir.ActivationFunctionType

    nc = bacc.Bacc("TRN2", target_bir_lowering=False, debug=False,
                   num_devices=N_CORES)

    xh_d = nc.dram_tensor("xh", [J, ROWS], F32, kind="ExternalInput")
    xhb_d = nc.dram_tensor("xhb", [J, ROWS], BF16, kind="ExternalInput")
    xlb_d = nc.dram_tensor("xlb", [J, ROWS], BF16, kind="ExternalInput")
    wh_d = nc.dram_tensor("wh", [J, KD, IC], F32, kind="ExternalInput")
    whb_d = nc.dram_tensor("whb", [KD, J, IC], BF16, kind="ExternalInput")
    wlb_d = nc.dram_tensor("wlb", [KD, J, IC], BF16, kind="ExternalInput")
    u0_d = nc.dram_tensor("u0", [IC, BH], F32, kind="ExternalInput")
    pch_d = nc.dram_tensor("pch", [IC, 3], F32, kind="ExternalInput")
    sout_d = nc.dram_tensor("sout", [IC, ROWS], F32, kind="ExternalOutput")

    with tile.TileContext(nc) as tc:
        with (
            tc.tile_pool(name="big", bufs=1) as big,
            tc.tile_pool(name="xs", bufs=3) as xs,
            tc.tile_pool(name="small", bufs=1) as small,
            tc.tile_pool(name="ps", bufs=4, space="PSUM") as ps,
            tc.tile_pool(name="dram", bufs=1, space="DRAM") as dram,
        ):
            At = [big.tile([IC, CHUNK], F32, tag=f"A{r}", name=f"A{r}")
                  for r in range(NCH)]
            Ut = [big.tile([IC, CHUNK], F32, tag=f"U{r}", name=f"U{r}")
                  for r in range(NCH)]
            scr = big.tile([IC, CHUNK], F32, tag="scr")
            whg = [small.tile([J, 4, IC], F32R, tag=f"whg{g}", name=f"whg{g}")
                   for g in range(4)]
            whb = small.tile([J, KD, IC], BF16, tag="whb")
            wlb = small.tile([J, KD, IC], BF16, tag="wlb")
            pch = small.tile([IC, 3], F32, tag="pch")
            Wc = small.tile([IC, BH], F32, tag="Wc")
            ssum = small.tile([IC, NCH], F32, tag="ssum")
            ssq = small.tile([IC, NCH], F32, tag="ssq")
            st2 = small.tile([IC, 2], F32, tag="st2")
            gs = small.tile([IC, 2], F32, tag="gs")
            prm = small.tile([IC, 8], F32, tag="prm")

            cc_in = dram.tile([IC, 2], F32)
            cc_out = dram.tile([2, IC, 2], F32)

            # startup: weights grouped 4 taps per DMA, split across the SP
            # and ACT queues so they land just ahead of tap consumption; the
            # first x slice rides SP first (tap 0 only needs the pad memset
            # + 272-col slice).
            xh_c0 = xs.tile([J, PAD + CHUNK], F32R, tag="xh_c")
            nc.vector.memset(xh_c0[:, :PAD].bitcast(F32), 0.0)
            nc.sync.dma_start(xh_c0[:, PAD:PAD + 272], xh_d[:, 0:272].bitcast(F32R))
            nc.sync.dma_start(xh_c0[:, PAD + 272:], xh_d[:, 272:CHUNK].bitcast(F32R))
            nc.sync.dma_start(whg[2][:], wh_d[:, 8:12, :].bitcast(F32R))
            nc.scalar.dma_start(whg[0][:], wh_d[:, 0:4, :].bitcast(F32R))
            nc.scalar.dma_start(whg[1][:], wh_d[:, 4:8, :].bitcast(F32R))
            nc.scalar.dma_start(whg[3][:], wh_d[:, 12:16, :].bitcast(F32R))
            nc.scalar.dma_start(Wc[:], u0_d[:])     # host sends W'_0 = U0 + 1
            nc.scalar.dma_start(pch[:], pch_d[:])
            # Dummy sqrt so the act-table pass picks sqrt_and_friends (the
            # one set holding Copy/Square/Sqrt/Identity) at t=0 instead of
            # reloading tables right before the first affine.
            nc.vector.memset(prm[:], 0.0)
            nc.scalar.sqrt(prm[:, 7:8], prm[:, 6:7])
            # onemb = 1 - beta depends only on pch: compute off the
            # post-collective critical path.
            nc.vector.tensor_scalar(prm[:, 4:5], pch[:, 0:1], -1.0, 1.0,
                                    OP.mult, OP.add)
            # PE p-state warmup: dummy matmuls on the zeroed pad region while
            # the first weights are still in flight, so the real pass-1
            # matmuls start at full clock (model needs ~3us of PE busy).
            ptd = ps.tile([IC, PAD], F32, tag="pt")
            for _ in range(6):
                nc.tensor.matmul(ptd[:], xh_c0[:, 0:IC], xh_c0[:, 0:PAD],
                                 start=True, stop=True)
            beta = pch[:, 0:1]
            gamma = pch[:, 1:2]
            bnbeta = pch[:, 2:3]

            # ---- conv pass 1: main fp32r term; doubles as the BN stats source ----
            for r in range(NCH):
                c0 = r * CHUNK - PAD
                if r == 0:
                    xh_c = xh_c0
                else:
                    xh_c = xs.tile([J, PAD + CHUNK], F32R, tag="xh_c")
                    nc.sync.dma_start(xh_c[:], xh_d[:, c0:c0 + PAD + CHUNK].bitcast(F32R))

                pt = ps.tile([IC, CHUNK], F32, tag="pt")
                for k in range(KD):
                    nc.tensor.matmul(pt[:], whg[k // 4][:, k % 4, :],
                                     xh_c[:, k * BH:k * BH + CHUNK],
                                     start=(k == 0), stop=(k == KD - 1))

                if r < NCH - 1:
                    nc.scalar.activation(At[r][:], pt[:], AF.Copy,
                                         accum_out=ssum[:, r:r + 1])
                else:
                    # last chunk: skip the Copy's accumulator drain (it
                    # serializes the Square behind a 187ns readback); DVE
                    # reduces the row sum from SBUF instead.
                    nc.scalar.activation(At[r][:], pt[:], AF.Copy)
                    nc.vector.tensor_reduce(ssum[:, r:r + 1], At[r][:],
                                            mybir.AxisListType.X, OP.add)
                nc.scalar.activation(scr[:], pt[:], AF.Square,
                                     accum_out=ssq[:, r:r + 1])

            # ---- BN stats allreduce over the b-half pair ----
            nc.vector.tensor_reduce(st2[:, 0:1], ssum[:], mybir.AxisListType.X, OP.add)
            nc.vector.tensor_reduce(st2[:, 1:2], ssq[:], mybir.AxisListType.X, OP.add)
            nc.sync.dma_start(cc_in[:], st2[:])
            # AllGather + local add: same result as AllReduce (order-proof
            # since add is commutative) at roughly half the fixed latency.
            nc.gpsimd.collective_compute(
                "AllGather", OP.bypass,
                replica_groups=[[0, 1], [2, 3], [4, 5], [6, 7]],
                ins=[cc_in.opt()], outs=[cc_out.opt()],
            )
            # On the ACT queue: a sync-queue DMA here would wait on the
            # collective semaphore and stall every pass-2 x DMA behind it.
            # ACT's own downstream (the affine) waits on the fold anyway.
            gs4 = small.tile([IC, 4], F32, tag="gs4")
            nc.scalar.dma_start(gs4[:].rearrange("p (g s) -> p g s", g=2),
                                cc_out[:, :, :].transpose([1, 0, 2]))

            inv_n = 1.0 / (T * B)
            mean = prm[:, 0:1]; ey2 = prm[:, 1:2]; var = prm[:, 2:3]
            inv = prm[:, 3:4]; onemb = prm[:, 4:5]; av = prm[:, 5:6]
            bv = prm[:, 6:7]; tmp = prm[:, 7:8]

            def fold_block():
                # fold BN + (1-beta) + scan's -beta into per-channel a, b'.
                # Emitted after a few pass-2 copies so the ACT sqrt doesn't
                # re-serialize them behind the collective. onemb was computed
                # at startup.
                nc.vector.tensor_tensor(gs[:], gs4[:, 0:2], gs4[:, 2:4], OP.add)
                nc.vector.tensor_scalar(mean, gs[:, 0:1], inv_n, None, OP.mult)
                nc.vector.tensor_scalar(ey2, gs[:, 1:2], inv_n, EPS,
                                        OP.mult, OP.add)    # E[y^2] + eps
                # var_neg = mean^2 - (E[y^2]+eps); sqrt applies scale=-1
                nc.vector.scalar_tensor_tensor(var, mean, mean, ey2,
                                               OP.mult, OP.subtract)
                nc.scalar.activation(tmp, var, AF.Sqrt, scale=-1.0)
                nc.vector.reciprocal(inv, tmp)
                nc.vector.tensor_tensor(inv, gamma, inv, OP.mult)   # gamma*rsqrt
                nc.vector.tensor_tensor(av, onemb, inv, OP.mult)    # a = (1-b)*g*rsqrt
                nc.vector.tensor_tensor(tmp, inv, mean, OP.mult)
                nc.vector.tensor_tensor(tmp, bnbeta, tmp, OP.subtract)
                nc.vector.tensor_tensor(bv, onemb, tmp, OP.mult)    # (1-b)*(bn_b - g*rsqrt*mean)
                nc.vector.tensor_tensor(bv, bv, beta, OP.subtract)  # b' = b - beta

            # ---- conv pass 2: bf16 cross terms + affine, racing the scan ----
            for k in range(KD):
                nc.sync.dma_start(whb[:, k, :], whb_d[k, :, :])
                nc.sync.dma_start(wlb[:, k, :], wlb_d[k, :, :])
            FOLD_AT = 3
            for r in range(NCH):
                c0 = r * CHUNK - PAD
                xhb_c = xs.tile([J, PAD + CHUNK], BF16, tag="xhb_c")
                xlb_c = xs.tile([J, PAD + CHUNK], BF16, tag="xlb_c")
                if r == 0:
                    nc.vector.memset(xhb_c[:, :PAD], 0.0)
                    nc.vector.memset(xlb_c[:, :PAD], 0.0)
                    nc.sync.dma_start(xhb_c[:, PAD:], xhb_d[:, 0:CHUNK])
                    nc.sync.dma_start(xlb_c[:, PAD:], xlb_d[:, 0:CHUNK])
                else:
                    nc.sync.dma_start(xhb_c[:], xhb_d[:, c0:c0 + PAD + CHUNK])
                    nc.sync.dma_start(xlb_c[:], xlb_d[:, c0:c0 + PAD + CHUNK])

                pt2 = ps.tile([IC, CHUNK], F32, tag="pt2")
                for k in range(KD):
                    nc.tensor.matmul(pt2[:], wlb[:, k, :], xhb_c[:, k * BH:k * BH + CHUNK],
                                     start=(k == 0), stop=False)
                for k in range(KD):
                    nc.tensor.matmul(pt2[:], whb[:, k, :], xlb_c[:, k * BH:k * BH + CHUNK],
                                     start=False, stop=(k == KD - 1))

                s2 = xs.tile([IC, CHUNK], F32, tag="s2")
                sl = At[r][:]
                nc.scalar.activation(s2[:], pt2[:], AF.Copy)
                nc.gpsimd.tensor_tensor(sl, sl, s2[:], OP.add)
                # A' = a*y + b' in one ACT op (per-partition scale/bias)
                if r == FOLD_AT:
                    fold_block()
                    # chunk 0's affine in halves so the scan can start on the
                    # first 16 steps while the rest is still being scaled
                    HC = CHUNK // 2
                    nc.scalar.activation(At[0][:, :HC], At[0][:, :HC],
                                         AF.Identity, bias=bv, scale=av)
                    nc.scalar.activation(At[0][:, HC:], At[0][:, HC:],
                                         AF.Identity, bias=bv, scale=av)
                    for rr in range(1, FOLD_AT + 1):
                        nc.scalar.activation(At[rr][:], At[rr][:], AF.Identity,
                                             bias=bv, scale=av)
                elif r > FOLD_AT:
                    nc.scalar.activation(sl, sl, AF.Identity, bias=bv, scale=av)

            # ---- LIF scan: 2 DVE ops per step per chain, spikes off-chain ----
            # W' = U - S + 1 lets the reset fold into one scalar_tensor_tensor:
            #   U  = beta*W' + A'     (b' above already absorbed the -beta)
            #   W' = (U < 1) + U
            HB = BH // 2
            for t in range(T):
                rt, lt = t // TPC, (t % TPC) * BH
                a0 = At[rt][:, lt:lt + HB]
                a1 = At[rt][:, lt + HB:lt + BH]
                u0_ = Ut[rt][:, lt:lt + HB]
                u1_ = Ut[rt][:, lt + HB:lt + BH]
                w0_, w1_ = Wc[:, :HB], Wc[:, HB:]
                nc.vector.scalar_tensor_tensor(u0_, w0_, beta, a0, OP.mult, OP.add)
                nc.vector.scalar_tensor_tensor(u1_, w1_, beta, a1, OP.mult, OP.add)
                nc.vector.scalar_tensor_tensor(w0_, u0_, 1.0, u0_, OP.is_lt, OP.add)
                nc.vector.scalar_tensor_tensor(w1_, u1_, 1.0, u1_, OP.is_lt, OP.add)

            # ---- bulk spike extraction on Pool + DMA out ----
            for r in range(NCH):
                if r < NCH - 1:
                    nc.gpsimd.tensor_scalar(At[r][:], Ut[r][:], 1.0, None, OP.is_ge)
                    nc.sync.dma_start(sout_d[:, r * CHUNK:(r + 1) * CHUNK], At[r][:])
                else:
                    # last chunk in shrinking slices so the final DMA after
                    # step 1023 is minimal
                    bnds = [0, 128, 256, 384, 448, 480, 496, 512]
                    for q in range(len(bnds) - 1):
                        a, b = r * CHUNK + bnds[q], r * CHUNK + bnds[q + 1]
                        la, lb = bnds[q], bnds[q + 1]
                        nc.gpsimd.tensor_scalar(At[r][:, la:lb],
                                                Ut[r][:, la:lb],
                                                1.0, None, OP.is_ge)
                        nc.sync.dma_start(sout_d[:, a:b], At[r][:, la:lb])

    nc.finalize()
    return nc


def _prep_inputs(x, delay_w, delay_P, beta, bn_gamma, bn_beta, U0):
    import ml_dtypes
    c = (delay_P.astype(np.float32) + KD // 2)
    k = np.arange(KD, dtype=np.float32)
    g = np.exp(-0.5 * ((k[None, None, :] - c[:, :, None]) / SIG) ** 2).astype(np.float32)
    g = g / (g.sum(-1, keepdims=True) + np.float32(1e-7))
    kern = (delay_w.astype(np.float32)[:, :, None] * g).astype(np.float32)  # (I,J,KD)

    kh = _to_fp32r(kern)
    kl = (kern - kh).astype(np.float32)
    xh = _to_fp32r(x)
    xl = (x - xh).astype(np.float32)

    wt_h = np.ascontiguousarray(kh.transpose(2, 1, 0))                     # (KD,J,I) f32
    wt_hb = wt_h.astype(ml_dtypes.bfloat16)
    wt_lb = np.ascontiguousarray(kl.transpose(2, 1, 0)).astype(ml_dtypes.bfloat16)
    wt_hj = np.ascontiguousarray(kh.transpose(1, 2, 0))                    # (J,KD,I) f32

    xt_h = np.ascontiguousarray(xh.transpose(2, 0, 1))                     # (J,T,B) f32
    xt_hb = xt_h.astype(ml_dtypes.bfloat16)
    xt_lb = np.ascontiguousarray(xl.transpose(2, 0, 1)).astype(ml_dtypes.bfloat16)

    in_maps = []
    for core in range(N_CORES):
        gi, hi = core // 2, core % 2
        isl = slice(gi * IC, (gi + 1) * IC)
        bsl = slice(hi * BH, (hi + 1) * BH)
        pch = np.stack([beta[isl], bn_gamma[isl], bn_beta[isl]], axis=1)
        in_maps.append({
            "xh": np.ascontiguousarray(xt_h[:, :, bsl]).reshape(J, ROWS),
            "xhb": np.ascontiguousarray(xt_hb[:, :, bsl]).reshape(J, ROWS),
            "xlb": np.ascontiguousarray(xt_lb[:, :, bsl]).reshape(J, ROWS),
            "wh": np.ascontiguousarray(wt_hj[:, :, isl]),
            "whb": np.ascontiguousarray(wt_hb[:, :, isl]),
            "wlb": np.ascontiguousarray(wt_lb[:, :, isl]),
            "u0": np.ascontiguousarray(U0[bsl, isl].T) + np.float32(1.0),
            "pch": np.ascontiguousarray(pch.astype(np.float32)),
        })
    return in_maps


def run_spmd(in_maps, **kwargs):
    from concourse.bass_utils import run_bass_kernel_spmd
    if "nc" not in _CACHE:
        _CACHE["nc"] = _build_nc()
    return run_bass_kernel_spmd(_CACHE["nc"], in_maps,
                                core_ids=list(range(N_CORES)), **kwargs)


def kernel(x, delay_w, delay_P, beta, bn_gamma, bn_beta, U0):
    in_maps = _prep_inputs(np.asarray(x, np.float32), np.asarray(delay_w, np.float32),
                           np.asarray(delay_P, np.float32), np.asarray(beta, np.float32),
                           np.asarray(bn_gamma, np.float32), np.asarray(bn_beta, np.float32),
                           np.asarray(U0, np.float32))
    res = run_spmd(in_maps)
    out = np.empty((T, B, I), np.float32)
    for core in range(N_CORES):
        gi, hi = core // 2, core % 2
        s = res.results[core]["sout"].reshape(IC, T, BH)
        out[:, hi * BH:(hi + 1) * BH, gi * IC:(gi + 1) * IC] = s.transpose(1, 2, 0)
    return out


# revision 38
# speedup vs baseline: 1.0039x; 1.0015x over previous
"""DelayLMLIFLayer Trainium2 kernel.

Pipeline per core (8 cores, 4-way I-shard x 2-way B-shard):
  1. Pass 1: DCLS delayed conv main term as 16 time-shifted PSUM-accumulated
     f32r matmuls per chunk; ACT drains PSUM into At (y_main) while
     accumulating BN sum/sum-of-squares per chunk. Startup DMAs are split
     across the SP and ACT queues so the first matmul is gated by
     max(weights, x) rather than their sum.
  2. BN stats: pairwise AllGather (b-half pairs) + local add; fold BN affine,
     (1-beta) input scale, and the scan's -beta constant into per-channel
     a, b'. The post-collective DMA rides the ACT queue (the SP queue would
     stall pass-2 x DMAs behind the collective wait), and the fold is emitted
     mid-pass-2 so its ACT sqrt doesn't re-serialize the PSUM copies.
  3. Pass 2 (races the scan): bf16 cross terms (xh@wl + xl@wh) -> ACT copy,
     Pool add into At, ACT affine At = a*At + b' (per-partition scale/bias;
     chunk 0 in halves so the scan starts sooner).
  4. LIF scan on DVE, 2 ops/step/chain (2 chains of 8 batch): with
     W' := U - S + 1 (host seeds W'_0 = U0 + 1),
       U_t  = beta*W'_{t-1} + A'_t         (A' = a*y + b', b' folds -beta)
       W'_t = (U_t < 1) + U_t
     Spikes leave the critical chain entirely: S = (U >= 1) is computed
     chunk-wise in bulk on Pool from the stored U history, written over At,
     then DMA'd out (last chunk in quarters to shorten the tail).
Host does layout transposes and the fp32r/bf16 splits; device time is what
counts. 490us predicted vs the 663us 3-op-scan baseline.
"""
import sys
sys.path.insert(0, '/opt/trn_rl_repo')

import numpy as np

T, B, J, I, KD = 1024, 32, 128, 512, 16
SIG = 0.5
EPS = 1e-5
N_CORES = 8
BH = B // 2          # batch elems per core (b-half)
IC = 128             # channels per core (I-chunk)
ROWS = T * BH        # free-dim rows per core
PAD = (KD - 1) * BH  # left zero pad columns (240)
CHUNK = 512          # psum tile free size
NCH = ROWS // CHUNK  # 32 row chunks
TPC = CHUNK // BH    # 32 timesteps per chunk

_CACHE = {}


def _to_fp32r(x):
    u = np.ascontiguousarray(x, np.float32).view(np.uint32).astype(np.uint64)
    rnd = ((u >> 12) & 1) + 0x7FF
    u = ((u + rnd) >> 12) << 12
    return (u & 0xFFFFFFFF).astype(np.uint32).view(np.float32)


def _build_nc():
    import concourse.bacc as bacc
    import concourse.mybir as mybir
    import concourse.tile as tile

    F32 = mybir.dt.float32
    F32R = mybir.dt.float32r
    BF16 = mybir.dt.bfloat16
    OP = mybir.AluOpType
    AF = mybir.ActivationFunctionType

    nc = bacc.Bacc("TRN2", target_bir_lowering=False, debug=False,
                   num_devices=N_CORES)

    xh_d = nc.dram_tensor("xh", [J, ROWS], F32, kind="ExternalInput")
    xhb_d = nc.dram_tensor("xhb", [J, ROWS], BF16, kind="ExternalInput")
    xlb_d = nc.dram_tensor("xlb", [J, ROWS], BF16, kind="ExternalInput")
    wh_d = nc.dram_tensor("wh", [J, KD, IC], F32, kind="ExternalInput")
    whb_d = nc.dram_tensor("whb", [KD, J, IC], BF16, kind="ExternalInput")
    wlb_d = nc.dram_tensor("wlb", [KD, J, IC], BF16, kind="ExternalInput")
    u0_d = nc.dram_tensor("u0", [IC, BH], F32, kind="ExternalInput")
    pch_d = nc.dram_tensor("pch", [IC, 3], F32, kind="ExternalInput")
    sout_d = nc.dram_tensor("sout", [IC, ROWS], F32, kind="ExternalOutput")

    with tile.TileContext(nc) as tc:
        with (
            tc.tile_pool(name="big", bufs=1) as big,
            tc.tile_pool(name="xs", bufs=3) as xs,
            tc.tile_pool(name="small", bufs=1) as small,
            tc.tile_pool(name="ps", bufs=4, space="PSUM") as ps,
            tc.tile_pool(name="dram", bufs=1, space="DRAM") as dram,
        ):
            At = [big.tile([IC, CHUNK], F32, tag=f"A{r}", name=f"A{r}")
                  for r in range(NCH)]
            Ut = [big.tile([IC, CHUNK], F32, tag=f"U{r}", name=f"U{r}")
                  for r in range(NCH)]
            scr = big.tile([IC, CHUNK], F32, tag="scr")
            whg = [small.tile([J, 4, IC], F32R, tag=f"whg{g}", name=f"whg{g}")
                   for g in range(4)]
            whb = small.tile([J, KD, IC], BF16, tag="whb")
            wlb = small.tile([J, KD, IC], BF16, tag="wlb")
            pch = small.tile([IC, 3], F32, tag="pch")
            Wc = small.tile([IC, BH], F32, tag="Wc")
            ssum = small.tile([IC, NCH], F32, tag="ssum")
            ssq = small.tile([IC, NCH], F32, tag="ssq")
            st2 = small.tile([IC, 2], F32, tag="st2")
            gs = small.tile([IC, 2], F32, tag="gs")
            prm = small.tile([IC, 8], F32, tag="prm")

            cc_in = dram.tile([IC, 2], F32)
            cc_out = dram.tile([2, IC, 2], F32)

            # startup: weights grouped 4 taps per DMA, split across the SP
            # and ACT queues so they land just ahead of tap consumption; the
            # first x slice rides SP first (tap 0 only needs the pad memset
            # + 272-col slice).
            xh_c0 = xs.tile([J, PAD + CHUNK], F32R, tag="xh_c")
            nc.vector.memset(xh_c0[:, :PAD].bitcast(F32), 0.0)
            nc.sync.dma_start(xh_c0[:, PAD:PAD + 272], xh_d[:, 0:272].bitcast(F32R))
            nc.sync.dma_start(xh_c0[:, PAD + 272:], xh_d[:, 272:CHUNK].bitcast(F32R))
            nc.sync.dma_start(whg[2][:], wh_d[:, 8:12, :].bitcast(F32R))
            nc.scalar.dma_start(whg[0][:], wh_d[:, 0:4, :].bitcast(F32R))
            nc.scalar.dma_start(whg[1][:], wh_d[:, 4:8, :].bitcast(F32R))
            nc.scalar.dma_start(whg[3][:], wh_d[:, 12:16, :].bitcast(F32R))
            nc.scalar.dma_start(Wc[:], u0_d[:])     # host sends W'_0 = U0 + 1
            nc.scalar.dma_start(pch[:], pch_d[:])
            # Dummy sqrt so the act-table pass picks sqrt_and_friends (the
            # one set holding Copy/Square/Sqrt/Identity) at t=0 instead of
            # reloading tables right before the first affine.
            nc.vector.memset(prm[:], 0.0)
            nc.scalar.sqrt(prm[:, 7:8], prm[:, 6:7])
            # onemb = 1 - beta depends only on pch: compute off the
            # post-collective critical path.
            nc.vector.tensor_scalar(prm[:, 4:5], pch[:, 0:1], -1.0, 1.0,
                                    OP.mult, OP.add)
            # PE p-state warmup: dummy matmuls on the zeroed pad region while
            # the first weights are still in flight, so the real pass-1
            # matmuls start at full clock (model needs ~3us of PE busy).
            ptd = ps.tile([IC, PAD], F32, tag="pt")
            for _ in range(6):
                nc.tensor.matmul(ptd[:], xh_c0[:, 0:IC], xh_c0[:, 0:PAD],
                                 start=True, stop=True)
            beta = pch[:, 0:1]
            gamma = pch[:, 1:2]
            bnbeta = pch[:, 2:3]

            # ---- conv pass 1: main fp32r term; doubles as the BN stats source ----
            for r in range(NCH):
                c0 = r * CHUNK - PAD
                if r == 0:
                    xh_c = xh_c0
                else:
                    xh_c = xs.tile([J, PAD + CHUNK], F32R, tag="xh_c")
                    nc.sync.dma_start(xh_c[:], xh_d[:, c0:c0 + PAD + CHUNK].bitcast(F32R))

                pt = ps.tile([IC, CHUNK], F32, tag="pt")
                for k in range(KD):
                    nc.tensor.matmul(pt[:], whg[k // 4][:, k % 4, :],
                                     xh_c[:, k * BH:k * BH + CHUNK],
                                     start=(k == 0), stop=(k == KD - 1))

                if r < NCH - 1:
                    nc.scalar.activation(At[r][:], pt[:], AF.Copy,
                                         accum_out=ssum[:, r:r + 1])
                else:
                    # last chunk: skip the Copy's accumulator drain (it
                    # serializes the Square behind a 187ns readback); DVE
                    # reduces the row sum from SBUF instead.
                    nc.scalar.activation(At[r][:], pt[:], AF.Copy)
                    nc.vector.tensor_reduce(ssum[:, r:r + 1], At[r][:],
                                            mybir.AxisListType.X, OP.add)
                nc.scalar.activation(scr[:], pt[:], AF.Square,
                                     accum_out=ssq[:, r:r + 1])

            # ---- BN stats allreduce over the b-half pair ----
            nc.vector.tensor_reduce(st2[:, 0:1], ssum[:], mybir.AxisListType.X, OP.add)
            nc.vector.tensor_reduce(st2[:, 1:2], ssq[:], mybir.AxisListType.X, OP.add)
            nc.sync.dma_start(cc_in[:], st2[:])
            # AllGather + local add: same result as AllReduce (order-proof
            # since add is commutative) at roughly half the fixed latency.
            nc.gpsimd.collective_compute(
                "AllGather", OP.bypass,
                replica_groups=[[0, 1], [2, 3], [4, 5], [6, 7]],
                ins=[cc_in.opt()], outs=[cc_out.opt()],
            )
            # On the ACT queue: a sync-queue DMA here would wait on the
            # collective semaphore and stall every pass-2 x DMA behind it.
            # ACT's own downstream (the affine) waits on the fold anyway.
            gs4 = small.tile([IC, 4], F32, tag="gs4")
            nc.scalar.dma_start(gs4[:].rearrange("p (g s) -> p g s", g=2),
                                cc_out[:, :, :].transpose([1, 0, 2]))

            inv_n = 1.0 / (T * B)
            mean = prm[:, 0:1]; ey2 = prm[:, 1:2]; var = prm[:, 2:3]
            inv = prm[:, 3:4]; onemb = prm[:, 4:5]; av = prm[:, 5:6]
            bv = prm[:, 6:7]; tmp = prm[:, 7:8]

            def fold_block():
                # fold BN + (1-beta) + scan's -beta into per-channel a, b'.
                # Emitted after a few pass-2 copies so the ACT sqrt doesn't
                # re-serialize them behind the collective. onemb was computed
                # at startup.
                nc.vector.tensor_tensor(gs[:], gs4[:, 0:2], gs4[:, 2:4], OP.add)
                nc.vector.tensor_scalar(mean, gs[:, 0:1], inv_n, None, OP.mult)
                nc.vector.tensor_scalar(ey2, gs[:, 1:2], inv_n, EPS,
                                        OP.mult, OP.add)    # E[y^2] + eps
                # var_neg = mean^2 - (E[y^2]+eps); sqrt applies scale=-1
                nc.vector.scalar_tensor_tensor(var, mean, mean, ey2,
                                               OP.mult, OP.subtract)
                nc.scalar.activation(tmp, var, AF.Sqrt, scale=-1.0)
                nc.vector.reciprocal(inv, tmp)
                nc.vector.tensor_tensor(inv, gamma, inv, OP.mult)   # gamma*rsqrt
                nc.vector.tensor_tensor(av, onemb, inv, OP.mult)    # a = (1-b)*g*rsqrt
                nc.vector.tensor_tensor(tmp, inv, mean, OP.mult)
                nc.vector.tensor_tensor(tmp, bnbeta, tmp, OP.subtract)
                nc.vector.tensor_tensor(bv, onemb, tmp, OP.mult)    # (1-b)*(bn_b - g*rsqrt*mean)
                nc.vector.tensor_tensor(bv, bv, beta, OP.subtract)  # b' = b - beta

            # ---- conv pass 2: bf16 cross terms + affine, racing the scan ----
            for k in range(KD):
                nc.sync.dma_start(whb[:, k, :], whb_d[k, :, :])
                nc.sync.dma_start(wlb[:, k, :], wlb_d[k, :, :])
            FOLD_AT = 3
            for r in range(NCH):
                c0 = r * CHUNK - PAD
                xhb_c = xs.tile([J, PAD + CHUNK], BF16, tag="xhb_c")
                xlb_c = xs.tile([J, PAD + CHUNK], BF16, tag="xlb_c")
                if r == 0:
                    nc.vector.memset(xhb_c[:, :PAD], 0.0)
                    nc.vector.memset(xlb_c[:, :PAD], 0.0)
                    nc.sync.dma_start(xhb_c[:, PAD:], xhb_d[:, 0:CHUNK])
                    nc.sync.dma_start(xlb_c[:, PAD:], xlb_d[:, 0:CHUNK])
                else:
                    nc.sync.dma_start(xhb_c[:], xhb_d[:, c0:c0 + PAD + CHUNK])
                    nc.sync.dma_start(xlb_c[:], xlb_d[:, c0:c0 + PAD + CHUNK])

                pt2 = ps.tile([IC, CHUNK], F32, tag="pt2")
                for k in range(KD):
                    nc.tensor.matmul(pt2[:], wlb[:, k, :], xhb_c[:, k * BH:k * BH + CHUNK],
                                     start=(k == 0), stop=False)
                for k in range(KD):
                    nc.tensor.matmul(pt2[:], whb[:, k, :], xlb_c[:, k * BH:k * BH + CHUNK],
                                     start=False, stop=(k == KD - 1))

                s2 = xs.tile([IC, CHUNK], F32, tag="s2")
                sl = At[r][:]
                nc.scalar.activation(s2[:], pt2[:], AF.Copy)
                nc.gpsimd.tensor_tensor(sl, sl, s2[:], OP.add)
                # A' = a*y + b' in one ACT op (per-partition scale/bias)
                if r == FOLD_AT:
                    fold_block()
                    # chunk 0's affine in halves so the scan can start on the
                    # first 16 steps while the rest is still being scaled
                    HC = CHUNK // 2
                    nc.scalar.activation(At[0][:, :HC], At[0][:, :HC],
                                         AF.Identity, bias=bv, scale=av)
                    nc.scalar.activation(At[0][:, HC:], At[0][:, HC:],
                                         AF.Identity, bias=bv, scale=av)
                    for rr in range(1, FOLD_AT + 1):
                        nc.scalar.activation(At[rr][:], At[rr][:], AF.Identity,
                                             bias=bv, scale=av)
                elif r > FOLD_AT:
                    nc.scalar.activation(sl, sl, AF.Identity, bias=bv, scale=av)

            # ---- LIF scan: 2 DVE ops per step per chain, spikes off-chain ----
            # W' = U - S + 1 lets the reset fold into one scalar_tensor_tensor:
            #   U  = beta*W' + A'     (b' above already absorbed the -beta)
            #   W' = (U < 1) + U
            HB = BH // 2
            for t in range(T):
                rt, lt = t // TPC, (t % TPC) * BH
                a0 = At[rt][:, lt:lt + HB]
                a1 = At[rt][:, lt + HB:lt + BH]
                u0_ = Ut[rt][:, lt:lt + HB]
                u1_ = Ut[rt][:, lt + HB:lt + BH]
                w0_, w1_ = Wc[:, :HB], Wc[:, HB:]
                nc.vector.scalar_tensor_tensor(u0_, w0_, beta, a0, OP.mult, OP.add)
                nc.vector.scalar_tensor_tensor(u1_, w1_, beta, a1, OP.mult, OP.add)
                nc.vector.scalar_tensor_tensor(w0_, u0_, 1.0, u0_, OP.is_lt, OP.add)
                nc.vector.scalar_tensor_tensor(w1_, u1_, 1.0, u1_, OP.is_lt, OP.add)

            # ---- bulk spike extraction on Pool + DMA out ----
            for r in range(NCH):
                if r < NCH - 1:
                    nc.gpsimd.tensor_scalar(At[r][:], Ut[r][:], 1.0, None, OP.is_ge)
                    nc.sync.dma_start(sout_d[:, r * CHUNK:(r + 1) * CHUNK], At[r][:])
                else:
                    # last chunk in eighths so the post-scan tail is tiny
                    Q = CHUNK // 8
                    for q in range(8):
                        nc.gpsimd.tensor_scalar(At[r][:, q * Q:(q + 1) * Q],
                                                Ut[r][:, q * Q:(q + 1) * Q],
                                                1.0, None, OP.is_ge)
                        nc.sync.dma_start(
                            sout_d[:, r * CHUNK + q * Q:r * CHUNK + (q + 1) * Q],
                            At[r][:, q * Q:(q + 1) * Q])

    nc.finalize()
    return nc


def _prep_inputs(x, delay_w, delay_P, beta, bn_gamma, bn_beta, U0):
    import ml_dtypes
    c = (delay_P.astype(np.float32) + KD // 2)
    k = np.arange(KD, dtype=np.float32)
    g = np.exp(-0.5 * ((k[None, None, :] - c[:, :, None]) / SIG) ** 2).astype(np.float32)
    g = g / (g.sum(-1, keepdims=True) + np.float32(1e-7))
    kern = (delay_w.astype(np.float32)[:, :, None] * g).astype(np.float32)  # (I,J,KD)

    kh = _to_fp32r(kern)
    kl = (kern - kh).astype(np.float32)
    xh = _to_fp32r(x)
    xl = (x - xh).astype(np.float32)

    wt_h = np.ascontiguousarray(kh.transpose(2, 1, 0))                     # (KD,J,I) f32
    wt_hb = wt_h.astype(ml_dtypes.bfloat16)
    wt_lb = np.ascontiguousarray(kl.transpose(2, 1, 0)).astype(ml_dtypes.bfloat16)
    wt_hj = np.ascontiguousarray(kh.transpose(1, 2, 0))                    # (J,KD,I) f32

    xt_h = np.ascontiguousarray(xh.transpose(2, 0, 1))                     # (J,T,B) f32
    xt_hb = xt_h.astype(ml_dtypes.bfloat16)
    xt_lb = np.ascontiguousarray(xl.transpose(2, 0, 1)).astype(ml_dtypes.bfloat16)

    in_maps = []
    for core in range(N_CORES):
        gi, hi = core // 2, core % 2
        isl = slice(gi * IC, (gi + 1) * IC)
        bsl = slice(hi * BH, (hi + 1) * BH)
        pch = np.stack([beta[isl], bn_gamma[isl], bn_beta[isl]], axis=1)
        in_maps.append({
            "xh": np.ascontiguousarray(xt_h[:, :, bsl]).reshape(J, ROWS),
            "xhb": np.ascontiguousarray(xt_hb[:, :, bsl]).reshape(J, ROWS),
            "xlb": np.ascontiguousarray(xt_lb[:, :, bsl]).reshape(J, ROWS),
            "wh": np.ascontiguousarray(wt_hj[:, :, isl]),
            "whb": np.ascontiguousarray(wt_hb[:, :, isl]),
            "wlb": np.ascontiguousarray(wt_lb[:, :, isl]),
            "u0": np.ascontiguousarray(U0[bsl, isl].T) + np.float32(1.0),
            "pch": np.ascontiguousarray(pch.astype(np.float32)),
        })
    return in_maps


def run_spmd(in_maps, **kwargs):
    from concourse.bass_utils import run_bass_kernel_spmd
    if "nc" not in _CACHE:
        _CACHE["nc"] = _build_nc()
    return run_bass_kernel_spmd(_CACHE["nc"], in_maps,
                                core_ids=list(range(N_CORES)), **kwargs)


def kernel(x, delay_w, delay_P, beta, bn_gamma, bn_beta, U0):
    in_maps = _prep_inputs(np.asarray(x, np.float32), np.asarray(delay_w, np.float32),
                           np.asarray(delay_P, np.float32), np.asarray(beta, np.float32),
                           np.asarray(bn_gamma, np.float32), np.asarray(bn_beta, np.float32),
                           np.asarray(U0, np.float32))
    res = run_spmd(in_maps)
    out = np.empty((T, B, I), np.float32)
    for core in range(N_CORES):
        gi, hi = core // 2, core % 2
        s = res.results[core]["sout"].reshape(IC, T, BH)
        out[:, hi * BH:(hi + 1) * BH, gi * IC:(gi + 1) * IC] = s.transpose(1, 2, 0)
    return out


# revision 39
# speedup vs baseline: 1.0043x; 1.0003x over previous
"""DelayLMLIFLayer Trainium2 kernel.

Pipeline per core (8 cores, 4-way I-shard x 2-way B-shard):
  1. Pass 1: DCLS delayed conv main term as 16 time-shifted PSUM-accumulated
     f32r matmuls per chunk; ACT drains PSUM into At (y_main) while
     accumulating BN sum/sum-of-squares per chunk. Startup DMAs are split
     across the SP and ACT queues so the first matmul is gated by
     max(weights, x) rather than their sum.
  2. BN stats: pairwise AllGather (b-half pairs) + local add; fold BN affine,
     (1-beta) input scale, and the scan's -beta constant into per-channel
     a, b'. The post-collective DMA rides the ACT queue (the SP queue would
     stall pass-2 x DMAs behind the collective wait), and the fold is emitted
     mid-pass-2 so its ACT sqrt doesn't re-serialize the PSUM copies.
  3. Pass 2 (races the scan): bf16 cross terms (xh@wl + xl@wh) -> ACT copy,
     Pool add into At, ACT affine At = a*At + b' (per-partition scale/bias;
     chunk 0 in halves so the scan starts sooner).
  4. LIF scan on DVE, 2 ops/step/chain (2 chains of 8 batch): with
     W' := U - S + 1 (host seeds W'_0 = U0 + 1),
       U_t  = beta*W'_{t-1} + A'_t         (A' = a*y + b', b' folds -beta)
       W'_t = (U_t < 1) + U_t
     Spikes leave the critical chain entirely: S = (U >= 1) is computed
     chunk-wise in bulk on Pool from the stored U history, written over At,
     then DMA'd out (last chunk in quarters to shorten the tail).
Host does layout transposes and the fp32r/bf16 splits; device time is what
counts. 490us predicted vs the 663us 3-op-scan baseline.
"""
import sys
sys.path.insert(0, '/opt/trn_rl_repo')

import numpy as np

T, B, J, I, KD = 1024, 32, 128, 512, 16
SIG = 0.5
EPS = 1e-5
N_CORES = 8
BH = B // 2          # batch elems per core (b-half)
IC = 128             # channels per core (I-chunk)
ROWS = T * BH        # free-dim rows per core
PAD = (KD - 1) * BH  # left zero pad columns (240)
CHUNK = 512          # psum tile free size
NCH = ROWS // CHUNK  # 32 row chunks
TPC = CHUNK // BH    # 32 timesteps per chunk

_CACHE = {}


def _to_fp32r(x):
    u = np.ascontiguousarray(x, np.float32).view(np.uint32).astype(np.uint64)
    rnd = ((u >> 12) & 1) + 0x7FF
    u = ((u + rnd) >> 12) << 12
    return (u & 0xFFFFFFFF).astype(np.uint32).view(np.float32)


def _build_nc():
    import concourse.bacc as bacc
    import concourse.mybir as mybir
    import concourse.tile as tile

    F32 = mybir.dt.float32
    F32R = mybir.dt.float32r
    BF16 = mybir.dt.bfloat16
    OP = mybir.AluOpType
    AF = mybir.ActivationFunctionType

    nc = bacc.Bacc("TRN2", target_bir_lowering=False, debug=False,
                   num_devices=N_CORES)

    xh_d = nc.dram_tensor("xh", [J, ROWS], F32, kind="ExternalInput")
    xhb_d = nc.dram_tensor("xhb", [J, ROWS], BF16, kind="ExternalInput")
    xlb_d = nc.dram_tensor("xlb", [J, ROWS], BF16, kind="ExternalInput")
    wh_d = nc.dram_tensor("wh", [J, KD, IC], F32, kind="ExternalInput")
    whb_d = nc.dram_tensor("whb", [KD, J, IC], BF16, kind="ExternalInput")
    wlb_d = nc.dram_tensor("wlb", [KD, J, IC], BF16, kind="ExternalInput")
    u0_d = nc.dram_tensor("u0", [IC, BH], F32, kind="ExternalInput")
    pch_d = nc.dram_tensor("pch", [IC, 3], F32, kind="ExternalInput")
    sout_d = nc.dram_tensor("sout", [IC, ROWS], F32, kind="ExternalOutput")

    with tile.TileContext(nc) as tc:
        with (
            tc.tile_pool(name="big", bufs=1) as big,
            tc.tile_pool(name="xs", bufs=3) as xs,
            tc.tile_pool(name="small", bufs=1) as small,
            tc.tile_pool(name="ps", bufs=4, space="PSUM") as ps,
            tc.tile_pool(name="dram", bufs=1, space="DRAM") as dram,
        ):
            At = [big.tile([IC, CHUNK], F32, tag=f"A{r}", name=f"A{r}")
                  for r in range(NCH)]
            Ut = [big.tile([IC, CHUNK], F32, tag=f"U{r}", name=f"U{r}")
                  for r in range(NCH)]
            scr = big.tile([IC, CHUNK], F32, tag="scr")
            whg = [small.tile([J, 4, IC], F32R, tag=f"whg{g}", name=f"whg{g}")
                   for g in range(4)]
            whb = small.tile([J, KD, IC], BF16, tag="whb")
            wlb = small.tile([J, KD, IC], BF16, tag="wlb")
            pch = small.tile([IC, 3], F32, tag="pch")
            Wc = small.tile([IC, BH], F32, tag="Wc")
            ssum = small.tile([IC, NCH], F32, tag="ssum")
            ssq = small.tile([IC, NCH], F32, tag="ssq")
            st2 = small.tile([IC, 2], F32, tag="st2")
            gs = small.tile([IC, 2], F32, tag="gs")
            prm = small.tile([IC, 8], F32, tag="prm")

            cc_in = dram.tile([IC, 2], F32)
            cc_out = dram.tile([2, IC, 2], F32)

            # startup: weights grouped 4 taps per DMA, split across the SP
            # and ACT queues so they land just ahead of tap consumption; the
            # first x slice rides SP first (tap 0 only needs the pad memset
            # + 272-col slice).
            xh_c0 = xs.tile([J, PAD + CHUNK], F32R, tag="xh_c")
            nc.vector.memset(xh_c0[:, :PAD].bitcast(F32), 0.0)
            nc.sync.dma_start(xh_c0[:, PAD:PAD + 272], xh_d[:, 0:272].bitcast(F32R))
            nc.sync.dma_start(xh_c0[:, PAD + 272:], xh_d[:, 272:CHUNK].bitcast(F32R))
            nc.sync.dma_start(whg[2][:], wh_d[:, 8:12, :].bitcast(F32R))
            nc.scalar.dma_start(whg[0][:], wh_d[:, 0:4, :].bitcast(F32R))
            nc.scalar.dma_start(whg[1][:], wh_d[:, 4:8, :].bitcast(F32R))
            nc.scalar.dma_start(whg[3][:], wh_d[:, 12:16, :].bitcast(F32R))
            nc.scalar.dma_start(Wc[:], u0_d[:])     # host sends W'_0 = U0 + 1
            nc.scalar.dma_start(pch[:], pch_d[:])
            # Dummy sqrt so the act-table pass picks sqrt_and_friends (the
            # one set holding Copy/Square/Sqrt/Identity) at t=0 instead of
            # reloading tables right before the first affine.
            nc.vector.memset(prm[:], 0.0)
            nc.scalar.sqrt(prm[:, 7:8], prm[:, 6:7])
            # onemb = 1 - beta depends only on pch: compute off the
            # post-collective critical path.
            nc.vector.tensor_scalar(prm[:, 4:5], pch[:, 0:1], -1.0, 1.0,
                                    OP.mult, OP.add)
            # PE p-state warmup: dummy matmuls on the zeroed pad region while
            # the first weights are still in flight, so the real pass-1
            # matmuls start at full clock (model needs ~3us of PE busy).
            ptd = ps.tile([IC, PAD], F32, tag="pt")
            for _ in range(6):
                nc.tensor.matmul(ptd[:], xh_c0[:, 0:IC], xh_c0[:, 0:PAD],
                                 start=True, stop=True)
            beta = pch[:, 0:1]
            gamma = pch[:, 1:2]
            bnbeta = pch[:, 2:3]

            # ---- conv pass 1: main fp32r term; doubles as the BN stats source ----
            for r in range(NCH):
                c0 = r * CHUNK - PAD
                if r == 0:
                    xh_c = xh_c0
                else:
                    xh_c = xs.tile([J, PAD + CHUNK], F32R, tag="xh_c")
                    nc.sync.dma_start(xh_c[:], xh_d[:, c0:c0 + PAD + CHUNK].bitcast(F32R))

                pt = ps.tile([IC, CHUNK], F32, tag="pt")
                for k in range(KD):
                    nc.tensor.matmul(pt[:], whg[k // 4][:, k % 4, :],
                                     xh_c[:, k * BH:k * BH + CHUNK],
                                     start=(k == 0), stop=(k == KD - 1))

                if r < NCH - 1:
                    nc.scalar.activation(At[r][:], pt[:], AF.Copy,
                                         accum_out=ssum[:, r:r + 1])
                else:
                    # last chunk: skip the Copy's accumulator drain (it
                    # serializes the Square behind a 187ns readback); DVE
                    # reduces the row sum from SBUF instead.
                    nc.scalar.activation(At[r][:], pt[:], AF.Copy)
                    nc.vector.tensor_reduce(ssum[:, r:r + 1], At[r][:],
                                            mybir.AxisListType.X, OP.add)
                nc.scalar.activation(scr[:], pt[:], AF.Square,
                                     accum_out=ssq[:, r:r + 1])

            # ---- BN stats allreduce over the b-half pair ----
            nc.vector.tensor_reduce(st2[:, 0:1], ssum[:], mybir.AxisListType.X, OP.add)
            nc.vector.tensor_reduce(st2[:, 1:2], ssq[:], mybir.AxisListType.X, OP.add)
            nc.sync.dma_start(cc_in[:], st2[:])
            # AllGather + local add: same result as AllReduce (order-proof
            # since add is commutative) at roughly half the fixed latency.
            nc.gpsimd.collective_compute(
                "AllGather", OP.bypass,
                replica_groups=[[0, 1], [2, 3], [4, 5], [6, 7]],
                ins=[cc_in.opt()], outs=[cc_out.opt()],
            )
            # On the ACT queue: a sync-queue DMA here would wait on the
            # collective semaphore and stall every pass-2 x DMA behind it.
            # ACT's own downstream (the affine) waits on the fold anyway.
            gs4 = small.tile([IC, 4], F32, tag="gs4")
            nc.scalar.dma_start(gs4[:].rearrange("p (g s) -> p g s", g=2),
                                cc_out[:, :, :].transpose([1, 0, 2]))

            inv_n = 1.0 / (T * B)
            mean = prm[:, 0:1]; ey2 = prm[:, 1:2]; var = prm[:, 2:3]
            inv = prm[:, 3:4]; onemb = prm[:, 4:5]; av = prm[:, 5:6]
            bv = prm[:, 6:7]; tmp = prm[:, 7:8]

            def fold_block():
                # fold BN + (1-beta) + scan's -beta into per-channel a, b'.
                # Emitted after a few pass-2 copies so the ACT sqrt doesn't
                # re-serialize them behind the collective. onemb was computed
                # at startup.
                nc.vector.tensor_tensor(gs[:], gs4[:, 0:2], gs4[:, 2:4], OP.add)
                nc.vector.tensor_scalar(mean, gs[:, 0:1], inv_n, None, OP.mult)
                nc.vector.tensor_scalar(ey2, gs[:, 1:2], inv_n, EPS,
                                        OP.mult, OP.add)    # E[y^2] + eps
                # var_neg = mean^2 - (E[y^2]+eps); sqrt applies scale=-1
                nc.vector.scalar_tensor_tensor(var, mean, mean, ey2,
                                               OP.mult, OP.subtract)
                nc.scalar.activation(tmp, var, AF.Sqrt, scale=-1.0)
                nc.vector.reciprocal(inv, tmp)
                nc.vector.tensor_tensor(inv, gamma, inv, OP.mult)   # gamma*rsqrt
                nc.vector.tensor_tensor(av, onemb, inv, OP.mult)    # a = (1-b)*g*rsqrt
                nc.vector.tensor_tensor(tmp, inv, mean, OP.mult)
                nc.vector.tensor_tensor(tmp, bnbeta, tmp, OP.subtract)
                nc.vector.tensor_tensor(bv, onemb, tmp, OP.mult)    # (1-b)*(bn_b - g*rsqrt*mean)
                nc.vector.tensor_tensor(bv, bv, beta, OP.subtract)  # b' = b - beta

            # ---- conv pass 2: bf16 cross terms + affine, racing the scan ----
            for k in range(KD):
                nc.sync.dma_start(whb[:, k, :], whb_d[k, :, :])
                nc.sync.dma_start(wlb[:, k, :], wlb_d[k, :, :])
            FOLD_AT = 3
            for r in range(NCH):
                c0 = r * CHUNK - PAD
                xhb_c = xs.tile([J, PAD + CHUNK], BF16, tag="xhb_c")
                xlb_c = xs.tile([J, PAD + CHUNK], BF16, tag="xlb_c")
                if r == 0:
                    nc.vector.memset(xhb_c[:, :PAD], 0.0)
                    nc.vector.memset(xlb_c[:, :PAD], 0.0)
                    nc.sync.dma_start(xhb_c[:, PAD:], xhb_d[:, 0:CHUNK])
                    nc.sync.dma_start(xlb_c[:, PAD:], xlb_d[:, 0:CHUNK])
                else:
                    nc.sync.dma_start(xhb_c[:], xhb_d[:, c0:c0 + PAD + CHUNK])
                    nc.sync.dma_start(xlb_c[:], xlb_d[:, c0:c0 + PAD + CHUNK])

                pt2 = ps.tile([IC, CHUNK], F32, tag="pt2")
                for k in range(KD):
                    nc.tensor.matmul(pt2[:], wlb[:, k, :], xhb_c[:, k * BH:k * BH + CHUNK],
                                     start=(k == 0), stop=False)
                for k in range(KD):
                    nc.tensor.matmul(pt2[:], whb[:, k, :], xlb_c[:, k * BH:k * BH + CHUNK],
                                     start=False, stop=(k == KD - 1))

                s2 = xs.tile([IC, CHUNK], F32, tag="s2")
                sl = At[r][:]
                nc.scalar.activation(s2[:], pt2[:], AF.Copy)
                nc.gpsimd.tensor_tensor(sl, sl, s2[:], OP.add)
                # A' = a*y + b' in one ACT op (per-partition scale/bias)
                if r == FOLD_AT:
                    fold_block()
                    # chunk 0's affine with a narrow head slice so the scan
                    # can start on the first 4 steps while the rest is still
                    # being scaled
                    HC = 64
                    nc.scalar.activation(At[0][:, :HC], At[0][:, :HC],
                                         AF.Identity, bias=bv, scale=av)
                    nc.scalar.activation(At[0][:, HC:], At[0][:, HC:],
                                         AF.Identity, bias=bv, scale=av)
                    for rr in range(1, FOLD_AT + 1):
                        nc.scalar.activation(At[rr][:], At[rr][:], AF.Identity,
                                             bias=bv, scale=av)
                elif r > FOLD_AT:
                    nc.scalar.activation(sl, sl, AF.Identity, bias=bv, scale=av)

            # ---- LIF scan: 2 DVE ops per step per chain, spikes off-chain ----
            # W' = U - S + 1 lets the reset fold into one scalar_tensor_tensor:
            #   U  = beta*W' + A'     (b' above already absorbed the -beta)
            #   W' = (U < 1) + U
            HB = BH // 2
            for t in range(T):
                rt, lt = t // TPC, (t % TPC) * BH
                a0 = At[rt][:, lt:lt + HB]
                a1 = At[rt][:, lt + HB:lt + BH]
                u0_ = Ut[rt][:, lt:lt + HB]
                u1_ = Ut[rt][:, lt + HB:lt + BH]
                w0_, w1_ = Wc[:, :HB], Wc[:, HB:]
                nc.vector.scalar_tensor_tensor(u0_, w0_, beta, a0, OP.mult, OP.add)
                nc.vector.scalar_tensor_tensor(u1_, w1_, beta, a1, OP.mult, OP.add)
                nc.vector.scalar_tensor_tensor(w0_, u0_, 1.0, u0_, OP.is_lt, OP.add)
                nc.vector.scalar_tensor_tensor(w1_, u1_, 1.0, u1_, OP.is_lt, OP.add)

            # ---- bulk spike extraction on Pool + DMA out ----
            for r in range(NCH):
                if r < NCH - 1:
                    nc.gpsimd.tensor_scalar(At[r][:], Ut[r][:], 1.0, None, OP.is_ge)
                    nc.sync.dma_start(sout_d[:, r * CHUNK:(r + 1) * CHUNK], At[r][:])
                else:
                    # last chunk in eighths so the post-scan tail is tiny
                    Q = CHUNK // 8
                    for q in range(8):
                        nc.gpsimd.tensor_scalar(At[r][:, q * Q:(q + 1) * Q],
                                                Ut[r][:, q * Q:(q + 1) * Q],
                                                1.0, None, OP.is_ge)
                        nc.sync.dma_start(
                            sout_d[:, r * CHUNK + q * Q:r * CHUNK + (q + 1) * Q],
                            At[r][:, q * Q:(q + 1) * Q])

    nc.finalize()
    return nc


def _prep_inputs(x, delay_w, delay_P, beta, bn_gamma, bn_beta, U0):
    import ml_dtypes
    c = (delay_P.astype(np.float32) + KD // 2)
    k = np.arange(KD, dtype=np.float32)
    g = np.exp(-0.5 * ((k[None, None, :] - c[:, :, None]) / SIG) ** 2).astype(np.float32)
    g = g / (g.sum(-1, keepdims=True) + np.float32(1e-7))
    kern = (delay_w.astype(np.float32)[:, :, None] * g).astype(np.float32)  # (I,J,KD)

    kh = _to_fp32r(kern)
    kl = (kern - kh).astype(np.float32)
    xh = _to_fp32r(x)
    xl = (x - xh).astype(np.float32)

    wt_h = np.ascontiguousarray(kh.transpose(2, 1, 0))                     # (KD,J,I) f32
    wt_hb = wt_h.astype(ml_dtypes.bfloat16)
    wt_lb = np.ascontiguousarray(kl.transpose(2, 1, 0)).astype(ml_dtypes.bfloat16)
    wt_hj = np.ascontiguousarray(kh.transpose(1, 2, 0))                    # (J,KD,I) f32

    xt_h = np.ascontiguousarray(xh.transpose(2, 0, 1))                     # (J,T,B) f32
    xt_hb = xt_h.astype(ml_dtypes.bfloat16)
    xt_lb = np.ascontiguousarray(xl.transpose(2, 0, 1)).astype(ml_dtypes.bfloat16)

    in_maps = []
    for core in range(N_CORES):
        gi, hi = core // 2, core % 2
        isl = slice(gi * IC, (gi + 1) * IC)
        bsl = slice(hi * BH, (hi + 1) * BH)
        pch = np.stack([beta[isl], bn_gamma[isl], bn_beta[isl]], axis=1)
        in_maps.append({
            "xh": np.ascontiguousarray(xt_h[:, :, bsl]).reshape(J, ROWS),
            "xhb": np.ascontiguousarray(xt_hb[:, :, bsl]).reshape(J, ROWS),
            "xlb": np.ascontiguousarray(xt_lb[:, :, bsl]).reshape(J, ROWS),
            "wh": np.ascontiguousarray(wt_hj[:, :, isl]),
            "whb": np.ascontiguousarray(wt_hb[:, :, isl]),
            "wlb": np.ascontiguousarray(wt_lb[:, :, isl]),
            "u0": np.ascontiguousarray(U0[bsl, isl].T) + np.float32(1.0),
            "pch": np.ascontiguousarray(pch.astype(np.float32)),
        })
    return in_maps


def run_spmd(in_maps, **kwargs):
    from concourse.bass_utils import run_bass_kernel_spmd
    if "nc" not in _CACHE:
        _CACHE["nc"] = _build_nc()
    return run_bass_kernel_spmd(_CACHE["nc"], in_maps,
                                core_ids=list(range(N_CORES)), **kwargs)


def kernel(x, delay_w, delay_P, beta, bn_gamma, bn_beta, U0):
    in_maps = _prep_inputs(np.asarray(x, np.float32), np.asarray(delay_w, np.float32),
                           np.asarray(delay_P, np.float32), np.asarray(beta, np.float32),
                           np.asarray(bn_gamma, np.float32), np.asarray(bn_beta, np.float32),
                           np.asarray(U0, np.float32))
    res = run_spmd(in_maps)
    out = np.empty((T, B, I), np.float32)
    for core in range(N_CORES):
        gi, hi = core // 2, core % 2
        s = res.results[core]["sout"].reshape(IC, T, BH)
        out[:, hi * BH:(hi + 1) * BH, gi * IC:(gi + 1) * IC] = s.transpose(1, 2, 0)
    return out


# revision 40
# speedup vs baseline: 1.0044x; 1.0001x over previous
"""DelayLMLIFLayer Trainium2 kernel.

Pipeline per core (8 cores, 4-way I-shard x 2-way B-shard):
  1. Pass 1: DCLS delayed conv main term as 16 time-shifted PSUM-accumulated
     f32r matmuls per chunk; ACT drains PSUM into At (y_main) while
     accumulating BN sum/sum-of-squares per chunk. Startup DMAs are split
     across the SP and ACT queues so the first matmul is gated by
     max(weights, x) rather than their sum.
  2. BN stats: pairwise AllGather (b-half pairs) + local add; fold BN affine,
     (1-beta) input scale, and the scan's -beta constant into per-channel
     a, b'. The post-collective DMA rides the ACT queue (the SP queue would
     stall pass-2 x DMAs behind the collective wait), and the fold is emitted
     mid-pass-2 so its ACT sqrt doesn't re-serialize the PSUM copies.
  3. Pass 2 (races the scan): bf16 cross terms (xh@wl + xl@wh) -> ACT copy,
     Pool add into At, ACT affine At = a*At + b' (per-partition scale/bias;
     chunk 0 in halves so the scan starts sooner).
  4. LIF scan on DVE, 2 ops/step/chain (2 chains of 8 batch): with
     W' := U - S + 1 (host seeds W'_0 = U0 + 1),
       U_t  = beta*W'_{t-1} + A'_t         (A' = a*y + b', b' folds -beta)
       W'_t = (U_t < 1) + U_t
     Spikes leave the critical chain entirely: S = (U >= 1) is computed
     chunk-wise in bulk on Pool from the stored U history, written over At,
     then DMA'd out (last chunk in quarters to shorten the tail).
Host does layout transposes and the fp32r/bf16 splits; device time is what
counts. 490us predicted vs the 663us 3-op-scan baseline.
"""
import sys
sys.path.insert(0, '/opt/trn_rl_repo')

import numpy as np

T, B, J, I, KD = 1024, 32, 128, 512, 16
SIG = 0.5
EPS = 1e-5
N_CORES = 8
BH = B // 2          # batch elems per core (b-half)
IC = 128             # channels per core (I-chunk)
ROWS = T * BH        # free-dim rows per core
PAD = (KD - 1) * BH  # left zero pad columns (240)
CHUNK = 512          # psum tile free size
NCH = ROWS // CHUNK  # 32 row chunks
TPC = CHUNK // BH    # 32 timesteps per chunk

_CACHE = {}


def _to_fp32r(x):
    u = np.ascontiguousarray(x, np.float32).view(np.uint32).astype(np.uint64)
    rnd = ((u >> 12) & 1) + 0x7FF
    u = ((u + rnd) >> 12) << 12
    return (u & 0xFFFFFFFF).astype(np.uint32).view(np.float32)


def _build_nc():
    import concourse.bacc as bacc
    import concourse.mybir as mybir
    import concourse.tile as tile

    F32 = mybir.dt.float32
    F32R = mybir.dt.float32r
    BF16 = mybir.dt.bfloat16
    OP = mybir.AluOpType
    AF = mybir.ActivationFunctionType

    nc = bacc.Bacc("TRN2", target_bir_lowering=False, debug=False,
                   num_devices=N_CORES)

    xh_d = nc.dram_tensor("xh", [J, ROWS], F32, kind="ExternalInput")
    xhb_d = nc.dram_tensor("xhb", [J, ROWS], BF16, kind="ExternalInput")
    xlb_d = nc.dram_tensor("xlb", [J, ROWS], BF16, kind="ExternalInput")
    wh_d = nc.dram_tensor("wh", [J, KD, IC], F32, kind="ExternalInput")
    whb_d = nc.dram_tensor("whb", [KD, J, IC], BF16, kind="ExternalInput")
    wlb_d = nc.dram_tensor("wlb", [KD, J, IC], BF16, kind="ExternalInput")
    u0_d = nc.dram_tensor("u0", [IC, BH], F32, kind="ExternalInput")
    pch_d = nc.dram_tensor("pch", [IC, 3], F32, kind="ExternalInput")
    sout_d = nc.dram_tensor("sout", [IC, ROWS], F32, kind="ExternalOutput")

    with tile.TileContext(nc) as tc:
        with (
            tc.tile_pool(name="big", bufs=1) as big,
            tc.tile_pool(name="xs", bufs=3) as xs,
            tc.tile_pool(name="small", bufs=1) as small,
            tc.tile_pool(name="ps", bufs=4, space="PSUM") as ps,
            tc.tile_pool(name="dram", bufs=1, space="DRAM") as dram,
        ):
            At = [big.tile([IC, CHUNK], F32, tag=f"A{r}", name=f"A{r}")
                  for r in range(NCH)]
            Ut = [big.tile([IC, CHUNK], F32, tag=f"U{r}", name=f"U{r}")
                  for r in range(NCH)]
            scr = big.tile([IC, CHUNK], F32, tag="scr")
            whg = [small.tile([J, 4, IC], F32R, tag=f"whg{g}", name=f"whg{g}")
                   for g in range(4)]
            whb = small.tile([J, KD, IC], BF16, tag="whb")
            wlb = small.tile([J, KD, IC], BF16, tag="wlb")
            pch = small.tile([IC, 3], F32, tag="pch")
            Wc = small.tile([IC, BH], F32, tag="Wc")
            ssum = small.tile([IC, NCH], F32, tag="ssum")
            ssq = small.tile([IC, NCH], F32, tag="ssq")
            st2 = small.tile([IC, 2], F32, tag="st2")
            gs = small.tile([IC, 2], F32, tag="gs")
            prm = small.tile([IC, 8], F32, tag="prm")

            cc_in = dram.tile([IC, 2], F32)
            cc_out = dram.tile([2, IC, 2], F32)

            # startup: weights grouped 4 taps per DMA, split across the SP
            # and ACT queues so they land just ahead of tap consumption; the
            # first x slice rides SP first (tap 0 only needs the pad memset
            # + 272-col slice).
            xh_c0 = xs.tile([J, PAD + CHUNK], F32R, tag="xh_c")
            nc.vector.memset(xh_c0[:, :PAD].bitcast(F32), 0.0)
            nc.sync.dma_start(xh_c0[:, PAD:PAD + 272], xh_d[:, 0:272].bitcast(F32R))
            nc.sync.dma_start(xh_c0[:, PAD + 272:], xh_d[:, 272:CHUNK].bitcast(F32R))
            nc.sync.dma_start(whg[2][:], wh_d[:, 8:12, :].bitcast(F32R))
            nc.scalar.dma_start(whg[0][:], wh_d[:, 0:4, :].bitcast(F32R))
            nc.scalar.dma_start(whg[1][:], wh_d[:, 4:8, :].bitcast(F32R))
            nc.scalar.dma_start(whg[3][:], wh_d[:, 12:16, :].bitcast(F32R))
            nc.scalar.dma_start(Wc[:], u0_d[:])     # host sends W'_0 = U0 + 1
            nc.scalar.dma_start(pch[:], pch_d[:])
            # Dummy sqrt so the act-table pass picks sqrt_and_friends (the
            # one set holding Copy/Square/Sqrt/Identity) at t=0 instead of
            # reloading tables right before the first affine.
            nc.vector.memset(prm[:], 0.0)
            nc.scalar.sqrt(prm[:, 7:8], prm[:, 6:7])
            # onemb = 1 - beta depends only on pch: compute off the
            # post-collective critical path.
            nc.vector.tensor_scalar(prm[:, 4:5], pch[:, 0:1], -1.0, 1.0,
                                    OP.mult, OP.add)
            # PE p-state warmup: dummy matmuls on the zeroed pad region while
            # the first weights are still in flight, so the real pass-1
            # matmuls start at full clock (model needs ~3us of PE busy).
            ptd = ps.tile([IC, PAD], F32, tag="pt")
            for _ in range(6):
                nc.tensor.matmul(ptd[:], xh_c0[:, 0:IC], xh_c0[:, 0:PAD],
                                 start=True, stop=True)
            beta = pch[:, 0:1]
            gamma = pch[:, 1:2]
            bnbeta = pch[:, 2:3]

            # ---- conv pass 1: main fp32r term; doubles as the BN stats source ----
            for r in range(NCH):
                c0 = r * CHUNK - PAD
                if r == 0:
                    xh_c = xh_c0
                else:
                    xh_c = xs.tile([J, PAD + CHUNK], F32R, tag="xh_c")
                    nc.sync.dma_start(xh_c[:], xh_d[:, c0:c0 + PAD + CHUNK].bitcast(F32R))

                pt = ps.tile([IC, CHUNK], F32, tag="pt")
                for k in range(KD):
                    nc.tensor.matmul(pt[:], whg[k // 4][:, k % 4, :],
                                     xh_c[:, k * BH:k * BH + CHUNK],
                                     start=(k == 0), stop=(k == KD - 1))

                if r < NCH - 1:
                    nc.scalar.activation(At[r][:], pt[:], AF.Copy,
                                         accum_out=ssum[:, r:r + 1])
                else:
                    # last chunk: skip the Copy's accumulator drain (it
                    # serializes the Square behind a 187ns readback); DVE
                    # reduces the row sum from SBUF instead.
                    nc.scalar.activation(At[r][:], pt[:], AF.Copy)
                    nc.vector.tensor_reduce(ssum[:, r:r + 1], At[r][:],
                                            mybir.AxisListType.X, OP.add)
                nc.scalar.activation(scr[:], pt[:], AF.Square,
                                     accum_out=ssq[:, r:r + 1])

            # ---- BN stats allreduce over the b-half pair ----
            nc.vector.tensor_reduce(st2[:, 0:1], ssum[:], mybir.AxisListType.X, OP.add)
            nc.vector.tensor_reduce(st2[:, 1:2], ssq[:], mybir.AxisListType.X, OP.add)
            nc.sync.dma_start(cc_in[:], st2[:])
            # AllGather + local add: same result as AllReduce (order-proof
            # since add is commutative) at roughly half the fixed latency.
            nc.gpsimd.collective_compute(
                "AllGather", OP.bypass,
                replica_groups=[[0, 1], [2, 3], [4, 5], [6, 7]],
                ins=[cc_in.opt()], outs=[cc_out.opt()],
            )
            # On the ACT queue: a sync-queue DMA here would wait on the
            # collective semaphore and stall every pass-2 x DMA behind it.
            # ACT's own downstream (the affine) waits on the fold anyway.
            gs4 = small.tile([IC, 4], F32, tag="gs4")
            nc.scalar.dma_start(gs4[:].rearrange("p (g s) -> p g s", g=2),
                                cc_out[:, :, :].transpose([1, 0, 2]))

            inv_n = 1.0 / (T * B)
            mean = prm[:, 0:1]; ey2 = prm[:, 1:2]; var = prm[:, 2:3]
            inv = prm[:, 3:4]; onemb = prm[:, 4:5]; av = prm[:, 5:6]
            bv = prm[:, 6:7]; tmp = prm[:, 7:8]

            def fold_block():
                # fold BN + (1-beta) + scan's -beta into per-channel a, b'.
                # Emitted after a few pass-2 copies so the ACT sqrt doesn't
                # re-serialize them behind the collective. onemb was computed
                # at startup.
                nc.vector.tensor_tensor(gs[:], gs4[:, 0:2], gs4[:, 2:4], OP.add)
                nc.vector.tensor_scalar(mean, gs[:, 0:1], inv_n, None, OP.mult)
                nc.vector.tensor_scalar(ey2, gs[:, 1:2], inv_n, EPS,
                                        OP.mult, OP.add)    # E[y^2] + eps
                # var_neg = mean^2 - (E[y^2]+eps); sqrt applies scale=-1
                nc.vector.scalar_tensor_tensor(var, mean, mean, ey2,
                                               OP.mult, OP.subtract)
                nc.scalar.activation(tmp, var, AF.Sqrt, scale=-1.0)
                nc.vector.reciprocal(inv, tmp)
                nc.vector.tensor_tensor(inv, gamma, inv, OP.mult)   # gamma*rsqrt
                nc.vector.tensor_tensor(av, onemb, inv, OP.mult)    # a = (1-b)*g*rsqrt
                # tmp = inv*mean - bn_b = -(bn_b - g*rsqrt*mean)
                nc.vector.scalar_tensor_tensor(tmp, inv, mean, bnbeta,
                                               OP.mult, OP.subtract)
                # bv = onemb*tmp + beta = -[(1-b)*(bn_b - g*r*mean) - beta] = -b'
                nc.vector.scalar_tensor_tensor(bv, tmp, onemb, beta,
                                               OP.mult, OP.add)
                nc.vector.tensor_scalar(bv, bv, -1.0, None, OP.mult)

            # ---- conv pass 2: bf16 cross terms + affine, racing the scan ----
            for k in range(KD):
                nc.sync.dma_start(whb[:, k, :], whb_d[k, :, :])
                nc.sync.dma_start(wlb[:, k, :], wlb_d[k, :, :])
            FOLD_AT = 3
            for r in range(NCH):
                c0 = r * CHUNK - PAD
                xhb_c = xs.tile([J, PAD + CHUNK], BF16, tag="xhb_c")
                xlb_c = xs.tile([J, PAD + CHUNK], BF16, tag="xlb_c")
                if r == 0:
                    nc.vector.memset(xhb_c[:, :PAD], 0.0)
                    nc.vector.memset(xlb_c[:, :PAD], 0.0)
                    nc.sync.dma_start(xhb_c[:, PAD:], xhb_d[:, 0:CHUNK])
                    nc.sync.dma_start(xlb_c[:, PAD:], xlb_d[:, 0:CHUNK])
                else:
                    nc.sync.dma_start(xhb_c[:], xhb_d[:, c0:c0 + PAD + CHUNK])
                    nc.sync.dma_start(xlb_c[:], xlb_d[:, c0:c0 + PAD + CHUNK])

                pt2 = ps.tile([IC, CHUNK], F32, tag="pt2")
                for k in range(KD):
                    nc.tensor.matmul(pt2[:], wlb[:, k, :], xhb_c[:, k * BH:k * BH + CHUNK],
                                     start=(k == 0), stop=False)
                for k in range(KD):
                    nc.tensor.matmul(pt2[:], whb[:, k, :], xlb_c[:, k * BH:k * BH + CHUNK],
                                     start=False, stop=(k == KD - 1))

                s2 = xs.tile([IC, CHUNK], F32, tag="s2")
                sl = At[r][:]
                nc.scalar.activation(s2[:], pt2[:], AF.Copy)
                nc.gpsimd.tensor_tensor(sl, sl, s2[:], OP.add)
                # A' = a*y + b' in one ACT op (per-partition scale/bias)
                if r == FOLD_AT:
                    fold_block()
                    # chunk 0's affine with a narrow head slice so the scan
                    # can start on the first 4 steps while the rest is still
                    # being scaled
                    HC = 64
                    nc.scalar.activation(At[0][:, :HC], At[0][:, :HC],
                                         AF.Identity, bias=bv, scale=av)
                    nc.scalar.activation(At[0][:, HC:], At[0][:, HC:],
                                         AF.Identity, bias=bv, scale=av)
                    for rr in range(1, FOLD_AT + 1):
                        nc.scalar.activation(At[rr][:], At[rr][:], AF.Identity,
                                             bias=bv, scale=av)
                elif r > FOLD_AT:
                    nc.scalar.activation(sl, sl, AF.Identity, bias=bv, scale=av)

            # ---- LIF scan: 2 DVE ops per step per chain, spikes off-chain ----
            # W' = U - S + 1 lets the reset fold into one scalar_tensor_tensor:
            #   U  = beta*W' + A'     (b' above already absorbed the -beta)
            #   W' = (U < 1) + U
            HB = BH // 2
            for t in range(T):
                rt, lt = t // TPC, (t % TPC) * BH
                a0 = At[rt][:, lt:lt + HB]
                a1 = At[rt][:, lt + HB:lt + BH]
                u0_ = Ut[rt][:, lt:lt + HB]
                u1_ = Ut[rt][:, lt + HB:lt + BH]
                w0_, w1_ = Wc[:, :HB], Wc[:, HB:]
                nc.vector.scalar_tensor_tensor(u0_, w0_, beta, a0, OP.mult, OP.add)
                nc.vector.scalar_tensor_tensor(u1_, w1_, beta, a1, OP.mult, OP.add)
                nc.vector.scalar_tensor_tensor(w0_, u0_, 1.0, u0_, OP.is_lt, OP.add)
                nc.vector.scalar_tensor_tensor(w1_, u1_, 1.0, u1_, OP.is_lt, OP.add)

            # ---- bulk spike extraction on Pool + DMA out ----
            for r in range(NCH):
                if r < NCH - 1:
                    nc.gpsimd.tensor_scalar(At[r][:], Ut[r][:], 1.0, None, OP.is_ge)
                    nc.sync.dma_start(sout_d[:, r * CHUNK:(r + 1) * CHUNK], At[r][:])
                else:
                    # last chunk in eighths so the post-scan tail is tiny
                    Q = CHUNK // 8
                    for q in range(8):
                        nc.gpsimd.tensor_scalar(At[r][:, q * Q:(q + 1) * Q],
                                                Ut[r][:, q * Q:(q + 1) * Q],
                                                1.0, None, OP.is_ge)
                        nc.sync.dma_start(
                            sout_d[:, r * CHUNK + q * Q:r * CHUNK + (q + 1) * Q],
                            At[r][:, q * Q:(q + 1) * Q])

    nc.finalize()
    return nc


def _prep_inputs(x, delay_w, delay_P, beta, bn_gamma, bn_beta, U0):
    import ml_dtypes
    c = (delay_P.astype(np.float32) + KD // 2)
    k = np.arange(KD, dtype=np.float32)
    g = np.exp(-0.5 * ((k[None, None, :] - c[:, :, None]) / SIG) ** 2).astype(np.float32)
    g = g / (g.sum(-1, keepdims=True) + np.float32(1e-7))
    kern = (delay_w.astype(np.float32)[:, :, None] * g).astype(np.float32)  # (I,J,KD)

    kh = _to_fp32r(kern)
    kl = (kern - kh).astype(np.float32)
    xh = _to_fp32r(x)
    xl = (x - xh).astype(np.float32)

    wt_h = np.ascontiguousarray(kh.transpose(2, 1, 0))                     # (KD,J,I) f32
    wt_hb = wt_h.astype(ml_dtypes.bfloat16)
    wt_lb = np.ascontiguousarray(kl.transpose(2, 1, 0)).astype(ml_dtypes.bfloat16)
    wt_hj = np.ascontiguousarray(kh.transpose(1, 2, 0))                    # (J,KD,I) f32

    xt_h = np.ascontiguousarray(xh.transpose(2, 0, 1))                     # (J,T,B) f32
    xt_hb = xt_h.astype(ml_dtypes.bfloat16)
    xt_lb = np.ascontiguousarray(xl.transpose(2, 0, 1)).astype(ml_dtypes.bfloat16)

    in_maps = []
    for core in range(N_CORES):
        gi, hi = core // 2, core % 2
        isl = slice(gi * IC, (gi + 1) * IC)
        bsl = slice(hi * BH, (hi + 1) * BH)
        pch = np.stack([beta[isl], bn_gamma[isl], bn_beta[isl]], axis=1)
        in_maps.append({
            "xh": np.ascontiguousarray(xt_h[:, :, bsl]).reshape(J, ROWS),
            "xhb": np.ascontiguousarray(xt_hb[:, :, bsl]).reshape(J, ROWS),
            "xlb": np.ascontiguousarray(xt_lb[:, :, bsl]).reshape(J, ROWS),
            "wh": np.ascontiguousarray(wt_hj[:, :, isl]),
            "whb": np.ascontiguousarray(wt_hb[:, :, isl]),
            "wlb": np.ascontiguousarray(wt_lb[:, :, isl]),
            "u0": np.ascontiguousarray(U0[bsl, isl].T) + np.float32(1.0),
            "pch": np.ascontiguousarray(pch.astype(np.float32)),
        })
    return in_maps


def run_spmd(in_maps, **kwargs):
    from concourse.bass_utils import run_bass_kernel_spmd
    if "nc" not in _CACHE:
        _CACHE["nc"] = _build_nc()
    return run_bass_kernel_spmd(_CACHE["nc"], in_maps,
                                core_ids=list(range(N_CORES)), **kwargs)


def kernel(x, delay_w, delay_P, beta, bn_gamma, bn_beta, U0):
    in_maps = _prep_inputs(np.asarray(x, np.float32), np.asarray(delay_w, np.float32),
                           np.asarray(delay_P, np.float32), np.asarray(beta, np.float32),
                           np.asarray(bn_gamma, np.float32), np.asarray(bn_beta, np.float32),
                           np.asarray(U0, np.float32))
    res = run_spmd(in_maps)
    out = np.empty((T, B, I), np.float32)
    for core in range(N_CORES):
        gi, hi = core // 2, core % 2
        s = res.results[core]["sout"].reshape(IC, T, BH)
        out[:, hi * BH:(hi + 1) * BH, gi * IC:(gi + 1) * IC] = s.transpose(1, 2, 0)
    return out
